# revision 19
# baseline (speedup 1.0000x reference)
"""Causal multi-head attention with RoPE on 8 Trainium2 NeuronCores.

Sharding: core c -> batch b = c // 2, head-group g = c % 2 (8 heads each).
Each core computes q/k/v projections for its 512 output dims, RoPE, causal
attention for its 8 heads, and a partial O-projection. Host sums the two
partial outputs per batch and transposes back.

Device layout notes:
  - All matmul operands are bitcast to float32r (full PE rate at N>=256,
    fp32 storage).
  - q/k are kept transposed [d, s] per head-pair tile [128, 2048]
    (head 2p on partitions 0..63, head 2p+1 on 64..127).
  - RoPE: q' = q * cos + swap(q) * sin_signed, where swap is an
    adjacent-partition-pair permutation done with a 128x128 permutation
    matmul; cos/sin tables arrive pre-expanded from the host.
  - Scores are computed transposed (keys on partitions) so softmax
    needs no DVE reductions: exp() goes straight from PSUM through the
    scalar engine, the denominator comes from a ones-column appended to V,
    and causal masking is a post-exp affine_select fill with 0.
  - v is stored naturally [s, d] with per-head interleaved ones columns
    ([128, 8*65] tiles) so PV lhsT slices are contiguous.
"""

import os
import numpy as np

import concourse.bass as bass
import concourse.tile as tile
from concourse import bacc, mybir
from concourse.bass_utils import run_bass_kernel_spmd

F32 = mybir.dt.float32
F32R = mybir.dt.float32r
MULT = mybir.AluOpType.mult
IS_GE = mybir.AluOpType.is_ge
EXP = mybir.ActivationFunctionType.Exp

P = 128          # partitions
S = 2048         # sequence length
D = 1024         # model dim
DK = 64          # head dim
HPC = 8          # heads per core
NPAIR = 4        # head pairs per core
KT = 8           # 128-row k-tiles of the contraction dim (D)
CH = 512         # i-chunk width (f32r wants moving dim >= 256)
NCH = S // CH    # 4 i-chunks
NJT = S // P     # 16 j-tiles

_CACHED_NC = None
LAST_RESULTS = None


def _r32(ap):
    return ap.bitcast(F32R)


def build_nc():
    nc = bacc.Bacc("TRN2", target_bir_lowering=False, debug=False)

    xT = nc.dram_tensor("xT", [D, S], F32, kind="ExternalInput").ap()
    wq = nc.dram_tensor("wq", [D, 512], F32, kind="ExternalInput").ap()
    wk = nc.dram_tensor("wk", [D, 512], F32, kind="ExternalInput").ap()
    wv = nc.dram_tensor("wv", [D, 512], F32, kind="ExternalInput").ap()
    wo = nc.dram_tensor("wo", [512, D], F32, kind="ExternalInput").ap()
    cosn = nc.dram_tensor("cosn", [P, S], F32, kind="ExternalInput").ap()
    sins = nc.dram_tensor("sins", [P, S], F32, kind="ExternalInput").ap()
    psw = nc.dram_tensor("psw", [P, P], F32, kind="ExternalInput").ap()
    out = nc.dram_tensor("out", [D, S], F32, kind="ExternalOutput").ap()

    xT3 = xT.rearrange("(kt p) s -> p kt s", p=P)
    wq3 = wq.rearrange("(kt p) o -> p kt o", p=P)
    wk3 = wk.rearrange("(kt p) o -> p kt o", p=P)
    wv3 = wv.rearrange("(kt p) o -> p kt o", p=P)
    wo3 = wo.rearrange("(pt p) o -> p pt o", p=P)

    with tile.TileContext(nc) as tc:
        with tc.tile_pool(name="persist", bufs=1) as persist:
            cos_sb = persist.tile([P, S], F32, tag="cos")
            sin_sb = persist.tile([P, S], F32, tag="sin")
            psw_sb = persist.tile([P, P], F32R, tag="psw")
            nc.sync.dma_start(cos_sb[:], cosn)
            nc.sync.dma_start(sin_sb[:], sins)
            nc.sync.dma_start(psw_sb[:], psw.bitcast(F32R))

            v_sb = [persist.tile([P, HPC * 65], F32R, name=f"v{jt}", tag=f"v{jt}") for jt in range(NJT)]
            ones8 = persist.tile([P, HPC], F32, tag="ones8")
            nc.vector.memset(ones8[:], 1.0)
            # touch Exp early so the ~2.7us ACT table load overlaps DMAs
            nc.scalar.activation(ones8[0:1, :], ones8[0:1, :], EXP, scale=0.0)
            att_sb = [persist.tile([P, S], F32R, name=f"att{p}", tag=f"att{p}") for p in range(NPAIR)]

            # ---- Phases 2+3 per head pair ----
            pair_ctx = [
                tc.tile_pool(name="work", bufs=2),
                tc.tile_pool(name="tmp", bufs=2),
                tc.tile_pool(name="expp", bufs=2),
                tc.tile_pool(name="pp23", bufs=1, space="PSUM"),
            ]
            work, tmp, expp, pp = [c.__enter__() for c in pair_ctx]

            def p2_prefetch(pair):
                st = {}
                st["q"] = work.tile([P, S], F32R, tag="qpair", name=f"q{pair}")
                st["k"] = work.tile([P, S], F32R, tag="kpair", name=f"k{pair}")
                st["wq"] = work.tile([P, KT, P], F32R, tag="wqp", name=f"wq{pair}", bufs=1)
                st["wk"] = work.tile([P, KT, P], F32R, tag="wkp", name=f"wk{pair}", bufs=1)
                osl = slice(pair * P, (pair + 1) * P)
                nc.sync.dma_start(st["wq"][:], wq3[:, :, osl].bitcast(F32R))
                nc.sync.dma_start(st["wk"][:], wk3[:, :, osl].bitcast(F32R))
                return st

            def p2_load_x(st, c):
                ssl = slice(c * CH, (c + 1) * CH)
                xsl = work.tile([P, KT, CH], F32R, tag="xsl", bufs=2)
                for kq in range(4):
                    nc.sync.dma_start(
                        xsl[:, 2 * kq:2 * kq + 2, :],
                        xT3[:, 2 * kq:2 * kq + 2, ssl].bitcast(F32R))
                st["xsl"] = xsl

            def p2_proj(st, c, which):
                # one tensor (q or k): 8-matmul projection burst + RoPE
                ssl = slice(c * CH, (c + 1) * CH)
                w_t = st["wq"] if which == "q" else st["wk"]
                dst = st["q"] if which == "q" else st["k"]
                xsl = st["xsl"]
                ps2 = pp.tile([P, 2, CH], F32, tag="ps2", bufs=1)
                for kt in range(KT):
                    nc.tensor.matmul(
                        ps2[:, 0, :], _r32(w_t[:, kt, :]), _r32(xsl[:, kt, :]),
                        start=(kt == 0), stop=(kt == KT - 1))
                raw = tmp.tile([P, CH], F32R, tag="raw")
                nc.scalar.copy(out=raw[:], in_=ps2[:, 0, :])
                nc.tensor.matmul(
                    ps2[:, 1, :], _r32(psw_sb[:]), _r32(raw[:]),
                    start=True, stop=True)
                tsin = tmp.tile([P, CH], F32, tag="tsin")
                nc.vector.tensor_tensor(tsin[:], ps2[:, 1, :], sin_sb[:, ssl], MULT)
                nc.vector.tensor_tensor(dst[:, ssl], raw[:], cos_sb[:, ssl], MULT)
                nc.vector.tensor_add(out=dst[:, ssl], in0=dst[:, ssl], in1=tsin[:])

            def p3_chunk(pair, st, c, hooks):
                # hooks: {jt_index: fn} emitted between jt iterations to
                # interleave next-pair projection bursts into the PE queue
                h0c, h1c = 65 * (2 * pair), 65 * (2 * pair + 1)
                q_sb, k_sb = st["q"], st["k"]
                ssl = slice(c * CH, (c + 1) * CH)
                psA = pp.tile([65, CH], F32, tag="pvA", bufs=1)
                psB = pp.tile([65, CH], F32, tag="pvB", bufs=1)
                njt = 4 * c + 4
                for jt in range(njt):
                    for fn in hooks.get(jt, ()):
                        fn()
                    start = max(0, (jt - 4 * c) * P)
                    w = CH - start
                    jsl = slice(jt * P, (jt + 1) * P)
                    isl = slice(c * CH + start, (c + 1) * CH)
                    sc = pp.tile([P, 2, CH], F32, tag="sc", bufs=2)
                    nc.tensor.matmul(
                        sc[:, 0, start:], k_sb[0:DK, jsl], q_sb[0:DK, isl],
                        start=True, stop=True, tile_position=(0, 0))
                    nc.tensor.matmul(
                        sc[:, 1, start:], k_sb[DK:P, jsl], q_sb[DK:P, isl],
                        start=True, stop=True, tile_position=(DK, 0))
                    ex = expp.tile([P, 2, CH], F32R, tag="exp")
                    nc.scalar.activation(
                        ex[:, :, start:], sc[:, :, start:], EXP, scale=0.125)
                    if jt >= 4 * c:
                        for hq in range(2):
                            nc.gpsimd.affine_select(
                                out=ex[:, hq, start:], in_=ex[:, hq, start:],
                                compare_op=IS_GE, fill=0.0,
                                base=c * CH + start - jt * P,
                                channel_multiplier=-1,
                                pattern=[[1, w]])
                    first, last = (jt == 0), (jt == njt - 1)
                    nc.tensor.matmul(
                        psA[:, start:], v_sb[jt][:, h0c:h0c + 65],
                        ex[:, 0, start:], start=first, stop=last)
                    nc.tensor.matmul(
                        psB[:, start:], v_sb[jt][:, h1c:h1c + 65],
                        ex[:, 1, start:], start=first, stop=last)
                for ps_, hoff in ((psA, 0), (psB, DK)):
                    d0 = tmp.tile([1, CH], F32, tag="d0")
                    nc.vector.tensor_copy(out=d0[:], in_=ps_[DK:DK + 1, :])
                    rcp = tmp.tile([1, CH], F32, tag="rcp")
                    nc.vector.reciprocal_approx_fast(out=rcp[:], in_=d0[:])
                    bc = tmp.tile([DK, CH], F32, tag="bc")
                    nc.gpsimd.partition_broadcast(bc[:], rcp[:], channels=DK)
                    nc.vector.tensor_tensor(
                        att_sb[pair][hoff:hoff + DK, ssl],
                        ps_[0:DK, :], bc[:], MULT)

            wo_box = {}

            def p4_group(ot, c):
                ssl = slice(c * CH, (c + 1) * CH)
                pso = pp.tile([P, CH], F32, tag="ps2", bufs=1)
                for p_ in range(NPAIR):
                    nc.tensor.matmul(
                        pso[:],
                        wo_box["wo"][:, p_, ot * P:(ot + 1) * P],
                        _r32(att_sb[p_][:, ssl]),
                        start=(p_ == 0), stop=(p_ == NPAIR - 1))
                ob = tmp.tile([P, CH], F32, tag="tsin")
                nc.vector.tensor_copy(out=ob[:], in_=pso[:])
                nc.sync.dma_start(out[ot * P:(ot + 1) * P, ssl], ob[:])

            def p1_vproj(st, c):
                # v projection for the 4 s-tiles of chunk c, all heads
                xsl = st["xsl"]
                for sl in range(4):
                    stt = 4 * c + sl
                    psv = pp.tile([P, 512], F32, tag="sc", bufs=2, name="psv")
                    for kt in range(KT):
                        nc.tensor.matmul(
                            psv[:],
                            _r32(xsl[:, kt, sl * P:(sl + 1) * P]),
                            _r32(wv_sb[:, kt, :]),
                            start=(kt == 0), stop=(kt == KT - 1))
                    v3 = v_sb[stt][:].rearrange("p (h e) -> p h e", e=65)
                    nc.vector.tensor_copy(
                        out=v3[:, :, 0:DK],
                        in_=psv[:].rearrange("p (h d) -> p h d", d=DK))
                    nc.vector.tensor_copy(
                        out=v3[:, :, DK:65], in_=ones8[:, :, None])

            # prologue: v projection + full P2 for pair 0, chunk-streamed
            wv_sb = work.tile([P, KT, 512], F32R, tag="wv", bufs=1)
            nc.sync.dma_start(wv_sb[:, 0:2, :], wv3[:, 0:2, :].bitcast(F32R))
            nc.sync.dma_start(wv_sb[:, 2:KT, :], wv3[:, 2:KT, :].bitcast(F32R))
            st_cur = p2_prefetch(0)
            for c in range(NCH):
                p2_load_x(st_cur, c)
                p1_vproj(st_cur, c)
                p2_proj(st_cur, c, "q")
                p2_proj(st_cur, c, "k")
            for pair in range(NPAIR):
                st_next = p2_prefetch(pair + 1) if pair + 1 < NPAIR else None
                if st_next is None:
                    # last pair: O-projection weights reuse the xsl slot
                    wo_box["wo"] = work.tile([P, NPAIR, D], F32R, tag="xsl", name="wo_sb")
                    nc.sync.dma_start(wo_box["wo"][:], wo3.bitcast(F32R))
                for c in range(NCH):
                    hooks = {}
                    njt = 4 * c + 4
                    if st_next is not None:
                        p2_load_x(st_next, c)
                        hooks[njt // 3] = [
                            lambda sn=st_next, cc=c: p2_proj(sn, cc, "q")]
                        hooks[max(njt // 3 + 1, 2 * njt // 3)] = [
                            lambda sn=st_next, cc=c: p2_proj(sn, cc, "k")]
                    elif c > 0:
                        # interleave O-projection of chunk c-1 into this chunk
                        npts = min(4, njt - 1)
                        for gi in range(8):
                            key = 1 + (gi % npts) * (njt - 1) // npts
                            hooks.setdefault(key, []).append(
                                lambda o=gi, cc=c - 1: p4_group(o, cc))
                    p3_chunk(pair, st_cur, c, hooks)
                st_cur = st_next
            for ot in range(D // P):
                p4_group(ot, NCH - 1)

            for c_ in reversed(pair_ctx):
                c_.__exit__(None, None, None)

    nc.compile()
    return nc


def _get_nc():
    global _CACHED_NC
    if _CACHED_NC is None:
        _CACHED_NC = build_nc()
    return _CACHED_NC


def make_in_maps(x, token_positions, Wq, Wk, Wv, Wo):
    x = np.asarray(x, dtype=np.float32)
    Wq = np.asarray(Wq, dtype=np.float32)
    Wk = np.asarray(Wk, dtype=np.float32)
    Wv = np.asarray(Wv, dtype=np.float32)
    Wo = np.asarray(Wo, dtype=np.float32)
    pos = np.asarray(token_positions).astype(np.float64)

    freq_idx = np.arange(0, DK, 2, dtype=np.float64)
    inv_freq = 1.0 / (10000.0 ** (freq_idx / DK))
    ang = pos[:, None] * inv_freq[None, :]          # [S, DK/2]
    cos_t = np.cos(ang).astype(np.float32).T        # [DK/2, S]
    sin_t = np.sin(ang).astype(np.float32).T

    pidx = (np.arange(P) % DK) // 2
    cosn = np.ascontiguousarray(cos_t[pidx, :])     # [128, S]
    sgn = np.where(np.arange(P) % 2 == 0, -1.0, 1.0).astype(np.float32)
    sins = np.ascontiguousarray(sin_t[pidx, :] * sgn[:, None])

    psw = np.zeros((P, P), dtype=np.float32)
    psw[np.arange(P), np.arange(P) ^ 1] = 1.0

    in_maps = []
    for core in range(8):
        b, g = core // 2, core % 2
        sl = slice(512 * g, 512 * g + 512)
        in_maps.append({
            "xT": np.ascontiguousarray(x[b].T),
            "wq": np.ascontiguousarray(Wq[sl, :].T),
            "wk": np.ascontiguousarray(Wk[sl, :].T),
            "wv": np.ascontiguousarray(Wv[sl, :].T),
            "wo": np.ascontiguousarray(Wo[:, sl].T),
            "cosn": cosn,
            "sins": sins,
            "psw": psw,
        })
    return in_maps


def kernel(x, token_positions, Wq, Wk, Wv, Wo):
    global LAST_RESULTS
    nc = _get_nc()
    in_maps = make_in_maps(x, token_positions, Wq, Wk, Wv, Wo)
    res = run_bass_kernel_spmd(nc, in_maps, list(range(8)))
    LAST_RESULTS = res
    B = x.shape[0]
    outp = np.empty((B, S, D), dtype=np.float32)
    for b in range(B):
        outp[b] = (res.results[2 * b]["out"] + res.results[2 * b + 1]["out"]).T
    return outp


# revision 20
# speedup vs baseline: 1.0931x; 1.0931x over previous
"""Causal multi-head attention with RoPE on 8 Trainium2 NeuronCores.

Sharding: core c -> batch b = c // 2, head-group g = c % 2 (8 heads each).
Each core computes q/k/v projections for its 512 output dims, RoPE, causal
attention for its 8 heads, and a partial O-projection. Host sums the two
partial outputs per batch and transposes back.

Device layout notes:
  - All matmul operands are bitcast to float32r (full PE rate at N>=256,
    fp32 storage).
  - q/k are kept transposed [d, s] per head-pair tile [128, 2048]
    (head 2p on partitions 0..63, head 2p+1 on 64..127).
  - RoPE: q' = q * cos + swap(q) * sin_signed, where swap is an
    adjacent-partition-pair permutation done with a 128x128 permutation
    matmul; cos/sin tables arrive pre-expanded from the host.
  - Scores are computed transposed (keys on partitions) so softmax
    needs no DVE reductions: exp() goes straight from PSUM through the
    scalar engine, the denominator comes from a ones-column appended to V,
    and causal masking is a post-exp affine_select fill with 0.
  - v is stored naturally [s, d] with per-head interleaved ones columns
    ([128, 8*65] tiles) so PV lhsT slices are contiguous.
"""

import os
import numpy as np

import concourse.bass as bass
import concourse.tile as tile
from concourse import bacc, mybir
from concourse.bass_utils import run_bass_kernel_spmd

F32 = mybir.dt.float32
F32R = mybir.dt.float32r
MULT = mybir.AluOpType.mult
IS_GE = mybir.AluOpType.is_ge
EXP = mybir.ActivationFunctionType.Exp

P = 128          # partitions
S = 2048         # sequence length
D = 1024         # model dim
DK = 64          # head dim
HPC = 8          # heads per core
NPAIR = 4        # head pairs per core
KT = 8           # 128-row k-tiles of the contraction dim (D)
CH = 512         # i-chunk width (f32r wants moving dim >= 256)
NCH = S // CH    # 4 i-chunks
NJT = S // P     # 16 j-tiles

_CACHED_NC = None
LAST_RESULTS = None


def _r32(ap):
    return ap.bitcast(F32R)


def build_nc():
    nc = bacc.Bacc("TRN2", target_bir_lowering=False, debug=False)

    xT = nc.dram_tensor("xT", [D, S], F32, kind="ExternalInput").ap()
    wq = nc.dram_tensor("wq", [D, 512], F32, kind="ExternalInput").ap()
    wk = nc.dram_tensor("wk", [D, 512], F32, kind="ExternalInput").ap()
    wv = nc.dram_tensor("wv", [D, 512], F32, kind="ExternalInput").ap()
    wo = nc.dram_tensor("wo", [512, D], F32, kind="ExternalInput").ap()
    cosn = nc.dram_tensor("cosn", [P, S], F32, kind="ExternalInput").ap()
    sins = nc.dram_tensor("sins", [P, S], F32, kind="ExternalInput").ap()
    psw = nc.dram_tensor("psw", [P, P], F32, kind="ExternalInput").ap()
    out = nc.dram_tensor("out", [D, S], F32, kind="ExternalOutput").ap()

    xT3 = xT.rearrange("(kt p) s -> p kt s", p=P)
    wq3 = wq.rearrange("(kt p) o -> p kt o", p=P)
    wk3 = wk.rearrange("(kt p) o -> p kt o", p=P)
    wv3 = wv.rearrange("(kt p) o -> p kt o", p=P)
    wo3 = wo.rearrange("(pt p) o -> p pt o", p=P)

    with tile.TileContext(nc) as tc:
        with tc.tile_pool(name="persist", bufs=1) as persist:
            cos_sb = persist.tile([P, S], F32, tag="cos")
            sin_sb = persist.tile([P, S], F32, tag="sin")
            psw_sb = persist.tile([P, P], F32R, tag="psw")
            nc.sync.dma_start(cos_sb[:], cosn)
            nc.sync.dma_start(sin_sb[:], sins)
            nc.sync.dma_start(psw_sb[:], psw.bitcast(F32R))

            v_sb = [persist.tile([P, HPC * 65], F32R, name=f"v{jt}", tag=f"v{jt}") for jt in range(NJT)]
            ones8 = persist.tile([P, HPC], F32, tag="ones8")
            nc.vector.memset(ones8[:], 1.0)
            # touch Exp early so the ~2.7us ACT table load overlaps DMAs
            nc.scalar.activation(ones8[0:1, :], ones8[0:1, :], EXP, scale=0.0)
            att_sb = [persist.tile([P, S], F32R, name=f"att{p}", tag=f"att{p}") for p in range(NPAIR)]

            # ---- Phase 1: V projection, all heads at once (N=512) ----
            with (
                tc.tile_pool(name="p1w", bufs=1) as p1w,
                tc.tile_pool(name="pp1", bufs=1, space="PSUM") as pp1,
            ):
                wv_sb = p1w.tile([P, KT, 512], F32R, tag="wv")
                nc.sync.dma_start(wv_sb[:, 0:2, :], wv3[:, 0:2, :].bitcast(F32R))
                nc.sync.dma_start(wv_sb[:, 2:KT, :], wv3[:, 2:KT, :].bitcast(F32R))
                xkt = [p1w.tile([P, S], F32R, name=f"xkt{kt}", tag=f"xkt{kt}") for kt in range(KT)]
                for kt in range(KT):
                    nc.sync.dma_start(
                        xkt[kt][:, 0:256], xT3[:, kt, 0:256].bitcast(F32R))
                    nc.sync.dma_start(
                        xkt[kt][:, 256:S], xT3[:, kt, 256:S].bitcast(F32R))
                for st in range(NJT):
                    ps = pp1.tile([P, 512], F32, tag="p1ps", bufs=2)
                    for kt in range(KT):
                        nc.tensor.matmul(
                            ps[:],
                            _r32(xkt[kt][:, st * P:(st + 1) * P]),
                            _r32(wv_sb[:, kt, :]),
                            start=(kt == 0),
                            stop=(kt == KT - 1),
                        )
                    v3 = v_sb[st][:].rearrange("p (h e) -> p h e", e=65)
                    nc.vector.tensor_copy(
                        out=v3[:, :, 0:DK],
                        in_=ps[:].rearrange("p (h d) -> p h d", d=DK),
                    )
                    nc.vector.tensor_copy(
                        out=v3[:, :, DK:65], in_=ones8[:, :, None])

            # ---- Phases 2+3 per head pair ----
            pair_ctx = [
                tc.tile_pool(name="work", bufs=2),
                tc.tile_pool(name="tmp", bufs=2),
                tc.tile_pool(name="expp", bufs=3),
                tc.tile_pool(name="pp23", bufs=1, space="PSUM"),
            ]
            work, tmp, expp, pp = [c.__enter__() for c in pair_ctx]

            def p2_prefetch(pair):
                st = {}
                st["q"] = work.tile([P, S], F32R, tag="qpair", name=f"q{pair}")
                st["k"] = work.tile([P, S], F32R, tag="kpair", name=f"k{pair}")
                st["wq"] = work.tile([P, KT, P], F32R, tag="wqp", name=f"wq{pair}", bufs=1)
                st["wk"] = work.tile([P, KT, P], F32R, tag="wkp", name=f"wk{pair}", bufs=1)
                osl = slice(pair * P, (pair + 1) * P)
                nc.sync.dma_start(st["wq"][:], wq3[:, :, osl].bitcast(F32R))
                nc.sync.dma_start(st["wk"][:], wk3[:, :, osl].bitcast(F32R))
                return st

            def p2_load_x(st, c):
                ssl = slice(c * CH, (c + 1) * CH)
                xsl = work.tile([P, KT, CH], F32R, tag="xsl", bufs=2)
                for kq in range(4):
                    nc.sync.dma_start(
                        xsl[:, 2 * kq:2 * kq + 2, :],
                        xT3[:, 2 * kq:2 * kq + 2, ssl].bitcast(F32R))
                st["xsl"] = xsl

            def p2_proj(st, c, which):
                # one tensor (q or k): 8-matmul projection burst + RoPE
                ssl = slice(c * CH, (c + 1) * CH)
                w_t = st["wq"] if which == "q" else st["wk"]
                dst = st["q"] if which == "q" else st["k"]
                xsl = st["xsl"]
                ps2 = pp.tile([P, 2, CH], F32, tag="ps2", bufs=1)
                for kt in range(KT):
                    nc.tensor.matmul(
                        ps2[:, 0, :], _r32(w_t[:, kt, :]), _r32(xsl[:, kt, :]),
                        start=(kt == 0), stop=(kt == KT - 1))
                raw = tmp.tile([P, CH], F32R, tag="raw")
                nc.scalar.copy(out=raw[:], in_=ps2[:, 0, :])
                nc.tensor.matmul(
                    ps2[:, 1, :], _r32(psw_sb[:]), _r32(raw[:]),
                    start=True, stop=True)
                tsin = tmp.tile([P, CH], F32, tag="tsin")
                nc.vector.tensor_tensor(tsin[:], ps2[:, 1, :], sin_sb[:, ssl], MULT)
                nc.vector.tensor_tensor(dst[:, ssl], raw[:], cos_sb[:, ssl], MULT)
                nc.vector.tensor_add(out=dst[:, ssl], in0=dst[:, ssl], in1=tsin[:])

            def p3_chunk(pair, st, c, hooks):
                # hooks: {jt_index: fn} emitted between jt iterations to
                # interleave next-pair projection bursts into the PE queue
                h0c, h1c = 65 * (2 * pair), 65 * (2 * pair + 1)
                q_sb, k_sb = st["q"], st["k"]
                ssl = slice(c * CH, (c + 1) * CH)
                psA = pp.tile([65, CH], F32, tag="pvA", bufs=1)
                psB = pp.tile([65, CH], F32, tag="pvB", bufs=1)
                njt = 4 * c + 4
                for jt in range(njt):
                    for fn in hooks.get(jt, ()):
                        fn()
                    start = max(0, (jt - 4 * c) * P)
                    w = CH - start
                    jsl = slice(jt * P, (jt + 1) * P)
                    isl = slice(c * CH + start, (c + 1) * CH)
                    sc = pp.tile([P, 2, CH], F32, tag="sc", bufs=2)
                    nc.tensor.matmul(
                        sc[:, 0, start:], k_sb[0:DK, jsl], q_sb[0:DK, isl],
                        start=True, stop=True, tile_position=(0, 0))
                    nc.tensor.matmul(
                        sc[:, 1, start:], k_sb[DK:P, jsl], q_sb[DK:P, isl],
                        start=True, stop=True, tile_position=(DK, 0))
                    ex = expp.tile([P, 2, CH], F32R, tag="exp")
                    nc.scalar.activation(
                        ex[:, :, start:], sc[:, :, start:], EXP, scale=0.125)
                    if jt >= 4 * c:
                        for hq in range(2):
                            nc.gpsimd.affine_select(
                                out=ex[:, hq, start:], in_=ex[:, hq, start:],
                                compare_op=IS_GE, fill=0.0,
                                base=c * CH + start - jt * P,
                                channel_multiplier=-1,
                                pattern=[[1, w]])
                    first, last = (jt == 0), (jt == njt - 1)
                    nc.tensor.matmul(
                        psA[:, start:], v_sb[jt][:, h0c:h0c + 65],
                        ex[:, 0, start:], start=first, stop=last)
                    nc.tensor.matmul(
                        psB[:, start:], v_sb[jt][:, h1c:h1c + 65],
                        ex[:, 1, start:], start=first, stop=last)
                for ps_, hoff in ((psA, 0), (psB, DK)):
                    d0 = tmp.tile([1, CH], F32, tag="d0")
                    nc.vector.tensor_copy(out=d0[:], in_=ps_[DK:DK + 1, :])
                    rcp = tmp.tile([1, CH], F32, tag="rcp")
                    nc.vector.reciprocal_approx_fast(out=rcp[:], in_=d0[:])
                    bc = tmp.tile([DK, CH], F32, tag="bc")
                    nc.gpsimd.partition_broadcast(bc[:], rcp[:], channels=DK)
                    nc.vector.tensor_tensor(
                        att_sb[pair][hoff:hoff + DK, ssl],
                        ps_[0:DK, :], bc[:], MULT)

            wo_box = {}

            def p4_group(ot, c):
                ssl = slice(c * CH, (c + 1) * CH)
                pso = pp.tile([P, CH], F32, tag="ps2", bufs=1)
                for p_ in range(NPAIR):
                    nc.tensor.matmul(
                        pso[:],
                        wo_box["wo"][:, p_, ot * P:(ot + 1) * P],
                        _r32(att_sb[p_][:, ssl]),
                        start=(p_ == 0), stop=(p_ == NPAIR - 1))
                ob = tmp.tile([P, CH], F32, tag="tsin")
                nc.vector.tensor_copy(out=ob[:], in_=pso[:])
                nc.sync.dma_start(out[ot * P:(ot + 1) * P, ssl], ob[:])

            # prologue: full P2 for pair 0
            st_cur = p2_prefetch(0)
            for c in range(NCH):
                p2_load_x(st_cur, c)
                p2_proj(st_cur, c, "q")
                p2_proj(st_cur, c, "k")
            for pair in range(NPAIR):
                st_next = p2_prefetch(pair + 1) if pair + 1 < NPAIR else None
                if st_next is None:
                    # last pair: O-projection weights reuse the xsl slot
                    wo_box["wo"] = work.tile([P, NPAIR, D], F32R, tag="xsl", name="wo_sb")
                    nc.sync.dma_start(wo_box["wo"][:], wo3.bitcast(F32R))
                if st_next is not None:
                    p2_load_x(st_next, 0)
                for c in range(NCH):
                    hooks = {}
                    njt = 4 * c + 4
                    if st_next is not None:
                        hooks[njt // 3] = [
                            lambda sn=st_next, cc=c: p2_proj(sn, cc, "q")]
                        hooks[max(njt // 3 + 1, 2 * njt // 3)] = [
                            lambda sn=st_next, cc=c: p2_proj(sn, cc, "k")]
                        if c + 1 < NCH:
                            hooks.setdefault(njt // 2, []).append(
                                lambda sn=st_next, cc=c + 1: p2_load_x(sn, cc))
                    elif c > 0:
                        # interleave O-projection of chunk c-1 into this chunk
                        npts = min(4, njt - 1)
                        for gi in range(8):
                            key = 1 + (gi % npts) * (njt - 1) // npts
                            hooks.setdefault(key, []).append(
                                lambda o=gi, cc=c - 1: p4_group(o, cc))
                    p3_chunk(pair, st_cur, c, hooks)
                st_cur = st_next
            for ot in range(D // P):
                p4_group(ot, NCH - 1)

            for c_ in reversed(pair_ctx):
                c_.__exit__(None, None, None)

    nc.compile()
    return nc


def _get_nc():
    global _CACHED_NC
    if _CACHED_NC is None:
        _CACHED_NC = build_nc()
    return _CACHED_NC


def make_in_maps(x, token_positions, Wq, Wk, Wv, Wo):
    x = np.asarray(x, dtype=np.float32)
    Wq = np.asarray(Wq, dtype=np.float32)
    Wk = np.asarray(Wk, dtype=np.float32)
    Wv = np.asarray(Wv, dtype=np.float32)
    Wo = np.asarray(Wo, dtype=np.float32)
    pos = np.asarray(token_positions).astype(np.float64)

    freq_idx = np.arange(0, DK, 2, dtype=np.float64)
    inv_freq = 1.0 / (10000.0 ** (freq_idx / DK))
    ang = pos[:, None] * inv_freq[None, :]          # [S, DK/2]
    cos_t = np.cos(ang).astype(np.float32).T        # [DK/2, S]
    sin_t = np.sin(ang).astype(np.float32).T

    pidx = (np.arange(P) % DK) // 2
    cosn = np.ascontiguousarray(cos_t[pidx, :])     # [128, S]
    sgn = np.where(np.arange(P) % 2 == 0, -1.0, 1.0).astype(np.float32)
    sins = np.ascontiguousarray(sin_t[pidx, :] * sgn[:, None])

    psw = np.zeros((P, P), dtype=np.float32)
    psw[np.arange(P), np.arange(P) ^ 1] = 1.0

    in_maps = []
    for core in range(8):
        b, g = core // 2, core % 2
        sl = slice(512 * g, 512 * g + 512)
        in_maps.append({
            "xT": np.ascontiguousarray(x[b].T),
            "wq": np.ascontiguousarray(Wq[sl, :].T),
            "wk": np.ascontiguousarray(Wk[sl, :].T),
            "wv": np.ascontiguousarray(Wv[sl, :].T),
            "wo": np.ascontiguousarray(Wo[:, sl].T),
            "cosn": cosn,
            "sins": sins,
            "psw": psw,
        })
    return in_maps


def kernel(x, token_positions, Wq, Wk, Wv, Wo):
    global LAST_RESULTS
    nc = _get_nc()
    in_maps = make_in_maps(x, token_positions, Wq, Wk, Wv, Wo)
    res = run_bass_kernel_spmd(nc, in_maps, list(range(8)))
    LAST_RESULTS = res
    B = x.shape[0]
    outp = np.empty((B, S, D), dtype=np.float32)
    for b in range(B):
        outp[b] = (res.results[2 * b]["out"] + res.results[2 * b + 1]["out"]).T
    return outp


# revision 21
# speedup vs baseline: 1.1666x; 1.0672x over previous
"""Causal multi-head attention with RoPE on 8 Trainium2 NeuronCores.

Sharding: core c -> batch b = c // 2, head-group g = c % 2 (8 heads each).
Each core computes q/k/v projections for its 512 output dims, RoPE, causal
attention for its 8 heads, and a partial O-projection. Host sums the two
partial outputs per batch and transposes back.

Device layout notes:
  - All matmul operands are bitcast to float32r (full PE rate at N>=256,
    fp32 storage).
  - q/k are kept transposed [d, s] per head-pair tile [128, 2048]
    (head 2p on partitions 0..63, head 2p+1 on 64..127).
  - RoPE: q' = q * cos + swap(q) * sin_signed, where swap is an
    adjacent-partition-pair permutation done with a 128x128 permutation
    matmul; cos/sin tables arrive pre-expanded from the host.
  - Scores are computed transposed (keys on partitions) so softmax
    needs no DVE reductions: exp() goes straight from PSUM through the
    scalar engine, the denominator comes from a ones-column appended to V,
    and causal masking is a post-exp affine_select fill with 0.
  - v is stored naturally [s, d] with per-head interleaved ones columns
    ([128, 8*65] tiles) so PV lhsT slices are contiguous.
"""

import os
import numpy as np

import concourse.bass as bass
import concourse.tile as tile
from concourse import bacc, mybir
from concourse.bass_utils import run_bass_kernel_spmd

F32 = mybir.dt.float32
F32R = mybir.dt.float32r
MULT = mybir.AluOpType.mult
IS_GE = mybir.AluOpType.is_ge
EXP = mybir.ActivationFunctionType.Exp

P = 128          # partitions
S = 2048         # sequence length
D = 1024         # model dim
DK = 64          # head dim
HPC = 8          # heads per core
NPAIR = 4        # head pairs per core
KT = 8           # 128-row k-tiles of the contraction dim (D)
CH = 512         # i-chunk width (f32r wants moving dim >= 256)
NCH = S // CH    # 4 i-chunks
NJT = S // P     # 16 j-tiles

_CACHED_NC = None
LAST_RESULTS = None


def _r32(ap):
    return ap.bitcast(F32R)


def build_nc():
    nc = bacc.Bacc("TRN2", target_bir_lowering=False, debug=False)

    xT = nc.dram_tensor("xT", [D, S], F32, kind="ExternalInput").ap()
    wq = nc.dram_tensor("wq", [D, 512], F32, kind="ExternalInput").ap()
    wk = nc.dram_tensor("wk", [D, 512], F32, kind="ExternalInput").ap()
    wv = nc.dram_tensor("wv", [D, 512], F32, kind="ExternalInput").ap()
    wo = nc.dram_tensor("wo", [512, D], F32, kind="ExternalInput").ap()
    cosn = nc.dram_tensor("cosn", [P, S], F32, kind="ExternalInput").ap()
    sins = nc.dram_tensor("sins", [P, S], F32, kind="ExternalInput").ap()
    psw = nc.dram_tensor("psw", [P, P], F32, kind="ExternalInput").ap()
    out = nc.dram_tensor("out", [D, S], F32, kind="ExternalOutput").ap()

    xT3 = xT.rearrange("(kt p) s -> p kt s", p=P)
    wq3 = wq.rearrange("(kt p) o -> p kt o", p=P)
    wk3 = wk.rearrange("(kt p) o -> p kt o", p=P)
    wv3 = wv.rearrange("(kt p) o -> p kt o", p=P)
    wo3 = wo.rearrange("(pt p) o -> p pt o", p=P)

    with tile.TileContext(nc) as tc:
        with tc.tile_pool(name="persist", bufs=1) as persist:
            cos_sb = persist.tile([P, S], F32, tag="cos")
            sin_sb = persist.tile([P, S], F32, tag="sin")
            psw_sb = persist.tile([P, P], F32R, tag="psw")
            nc.sync.dma_start(cos_sb[:], cosn)
            nc.sync.dma_start(sin_sb[:], sins)
            nc.sync.dma_start(psw_sb[:], psw.bitcast(F32R))

            v_sb = [persist.tile([P, HPC * 65], F32R, name=f"v{jt}", tag=f"v{jt}") for jt in range(NJT)]
            ones8 = persist.tile([P, HPC], F32, tag="ones8")
            nc.vector.memset(ones8[:], 1.0)
            # touch Exp early so the ~2.7us ACT table load overlaps DMAs
            nc.scalar.activation(ones8[0:1, :], ones8[0:1, :], EXP, scale=0.0)
            att_sb = [persist.tile([P, S], F32R, name=f"att{p}", tag=f"att{p}") for p in range(NPAIR)]

            # ---- Phase 1: V projection, all heads at once (N=512) ----
            with (
                tc.tile_pool(name="p1w", bufs=1) as p1w,
                tc.tile_pool(name="pp1", bufs=1, space="PSUM") as pp1,
            ):
                wv_sb = p1w.tile([P, KT, 512], F32R, tag="wv")
                nc.sync.dma_start(wv_sb[:, 0:2, :], wv3[:, 0:2, :].bitcast(F32R))
                nc.sync.dma_start(wv_sb[:, 2:KT, :], wv3[:, 2:KT, :].bitcast(F32R))
                xkt = [p1w.tile([P, S], F32R, name=f"xkt{kt}", tag=f"xkt{kt}") for kt in range(KT)]
                for kt in range(KT):
                    nc.sync.dma_start(
                        xkt[kt][:, 0:256], xT3[:, kt, 0:256].bitcast(F32R))
                    nc.sync.dma_start(
                        xkt[kt][:, 256:S], xT3[:, kt, 256:S].bitcast(F32R))
                for st in range(NJT):
                    ps = pp1.tile([P, 512], F32, tag="p1ps", bufs=2)
                    for kt in range(KT):
                        nc.tensor.matmul(
                            ps[:],
                            _r32(xkt[kt][:, st * P:(st + 1) * P]),
                            _r32(wv_sb[:, kt, :]),
                            start=(kt == 0),
                            stop=(kt == KT - 1),
                        )
                    v3 = v_sb[st][:].rearrange("p (h e) -> p h e", e=65)
                    nc.vector.tensor_copy(
                        out=v3[:, :, 0:DK],
                        in_=ps[:].rearrange("p (h d) -> p h d", d=DK),
                    )
                    nc.vector.tensor_copy(
                        out=v3[:, :, DK:65], in_=ones8[:, :, None])

            # ---- Phases 2+3 per head pair ----
            pair_ctx = [
                tc.tile_pool(name="work", bufs=2),
                tc.tile_pool(name="tmp", bufs=2),
                tc.tile_pool(name="expp", bufs=3),
                tc.tile_pool(name="pp23", bufs=1, space="PSUM"),
            ]
            work, tmp, expp, pp = [c.__enter__() for c in pair_ctx]

            def p2_prefetch(pair):
                st = {}
                st["q"] = work.tile([P, S], F32R, tag="qpair", name=f"q{pair}")
                st["k"] = work.tile([P, S], F32R, tag="kpair", name=f"k{pair}")
                st["wq"] = work.tile([P, KT, P], F32R, tag="wqp", name=f"wq{pair}", bufs=1)
                st["wk"] = work.tile([P, KT, P], F32R, tag="wkp", name=f"wk{pair}", bufs=1)
                osl = slice(pair * P, (pair + 1) * P)
                nc.sync.dma_start(st["wq"][:], wq3[:, :, osl].bitcast(F32R))
                nc.sync.dma_start(st["wk"][:], wk3[:, :, osl].bitcast(F32R))
                return st

            def p2_load_x(st, c):
                ssl = slice(c * CH, (c + 1) * CH)
                xsl = work.tile([P, KT, CH], F32R, tag="xsl", bufs=2)
                for kq in range(4):
                    nc.sync.dma_start(
                        xsl[:, 2 * kq:2 * kq + 2, :],
                        xT3[:, 2 * kq:2 * kq + 2, ssl].bitcast(F32R))
                st["xsl"] = xsl

            def p2_proj(st, c, which):
                # one tensor (q or k): 8-matmul projection burst + RoPE
                ssl = slice(c * CH, (c + 1) * CH)
                w_t = st["wq"] if which == "q" else st["wk"]
                dst = st["q"] if which == "q" else st["k"]
                xsl = st["xsl"]
                ps2 = pp.tile([P, 2, CH], F32, tag="ps2", bufs=1)
                for kt in range(KT):
                    nc.tensor.matmul(
                        ps2[:, 0, :], _r32(w_t[:, kt, :]), _r32(xsl[:, kt, :]),
                        start=(kt == 0), stop=(kt == KT - 1))
                raw = tmp.tile([P, CH], F32R, tag="raw")
                nc.scalar.copy(out=raw[:], in_=ps2[:, 0, :])
                nc.tensor.matmul(
                    ps2[:, 1, :], _r32(psw_sb[:]), _r32(raw[:]),
                    start=True, stop=True)
                tsin = tmp.tile([P, CH], F32, tag="tsin")
                nc.vector.tensor_tensor(tsin[:], ps2[:, 1, :], sin_sb[:, ssl], MULT)
                nc.vector.tensor_tensor(dst[:, ssl], raw[:], cos_sb[:, ssl], MULT)
                nc.vector.tensor_add(out=dst[:, ssl], in0=dst[:, ssl], in1=tsin[:])

            def p3_chunk(pair, st, c, hooks):
                # hooks: {jt_index: fn} emitted between jt iterations to
                # interleave next-pair projection bursts into the PE queue
                h0c, h1c = 65 * (2 * pair), 65 * (2 * pair + 1)
                q_sb, k_sb = st["q"], st["k"]
                ssl = slice(c * CH, (c + 1) * CH)
                psA = pp.tile([65, CH], F32, tag="pvA", bufs=1)
                psB = pp.tile([65, CH], F32, tag="pvB", bufs=1)
                njt = 4 * c + 4
                for jt in range(njt):
                    for fn in hooks.get(jt, ()):
                        fn()
                    start = max(0, (jt - 4 * c) * P)
                    w = CH - start
                    jsl = slice(jt * P, (jt + 1) * P)
                    isl = slice(c * CH + start, (c + 1) * CH)
                    sc = pp.tile([P, 2, CH], F32, tag="sc", bufs=2)
                    nc.tensor.matmul(
                        sc[:, 0, start:], k_sb[0:DK, jsl], q_sb[0:DK, isl],
                        start=True, stop=True, tile_position=(0, 0))
                    nc.tensor.matmul(
                        sc[:, 1, start:], k_sb[DK:P, jsl], q_sb[DK:P, isl],
                        start=True, stop=True, tile_position=(DK, 0))
                    ex = expp.tile([P, 2, CH], F32R, tag="exp")
                    nc.scalar.activation(
                        ex[:, :, start:], sc[:, :, start:], EXP, scale=0.125)
                    if jt >= 4 * c:
                        for hq in range(2):
                            nc.gpsimd.affine_select(
                                out=ex[:, hq, start:], in_=ex[:, hq, start:],
                                compare_op=IS_GE, fill=0.0,
                                base=c * CH + start - jt * P,
                                channel_multiplier=-1,
                                pattern=[[1, w]])
                    first, last = (jt == 0), (jt == njt - 1)
                    nc.tensor.matmul(
                        psA[:, start:], v_sb[jt][:, h0c:h0c + 65],
                        ex[:, 0, start:], start=first, stop=last)
                    nc.tensor.matmul(
                        psB[:, start:], v_sb[jt][:, h1c:h1c + 65],
                        ex[:, 1, start:], start=first, stop=last)
                for ps_, hoff in ((psA, 0), (psB, DK)):
                    d0 = tmp.tile([1, CH], F32, tag="d0")
                    nc.vector.tensor_copy(out=d0[:], in_=ps_[DK:DK + 1, :])
                    rcp = tmp.tile([1, CH], F32, tag="rcp")
                    nc.vector.reciprocal_approx_fast(out=rcp[:], in_=d0[:])
                    bc = tmp.tile([DK, CH], F32, tag="bc")
                    nc.gpsimd.partition_broadcast(bc[:], rcp[:], channels=DK)
                    nc.vector.tensor_tensor(
                        att_sb[pair][hoff:hoff + DK, ssl],
                        ps_[0:DK, :], bc[:], MULT)

            wo_box = {}

            def p4_group(ot, c):
                ssl = slice(c * CH, (c + 1) * CH)
                pso = pp.tile([P, CH], F32, tag="ps2", bufs=1)
                for p_ in range(NPAIR):
                    nc.tensor.matmul(
                        pso[:],
                        wo_box["wo"][:, p_, ot * P:(ot + 1) * P],
                        _r32(att_sb[p_][:, ssl]),
                        start=(p_ == 0), stop=(p_ == NPAIR - 1))
                ob = tmp.tile([P, CH], F32, tag="tsin")
                nc.vector.tensor_copy(out=ob[:], in_=pso[:])
                nc.sync.dma_start(out[ot * P:(ot + 1) * P, ssl], ob[:])

            # prologue: full P2 for pair 0
            st_cur = p2_prefetch(0)
            for c in range(NCH):
                p2_load_x(st_cur, c)
                p2_proj(st_cur, c, "q")
                p2_proj(st_cur, c, "k")
            for pair in range(NPAIR):
                st_next = p2_prefetch(pair + 1) if pair + 1 < NPAIR else None
                if st_next is None:
                    # last pair: O-projection weights reuse the xsl slot
                    wo_box["wo"] = work.tile([P, NPAIR, D], F32R, tag="xsl", name="wo_sb")
                    nc.sync.dma_start(wo_box["wo"][:], wo3.bitcast(F32R))
                for c in range(NCH):
                    hooks = {}
                    njt = 4 * c + 4
                    if st_next is not None:
                        p2_load_x(st_next, c)
                        hooks[njt // 3] = [
                            lambda sn=st_next, cc=c: p2_proj(sn, cc, "q")]
                        hooks[max(njt // 3 + 1, 2 * njt // 3)] = [
                            lambda sn=st_next, cc=c: p2_proj(sn, cc, "k")]
                    elif c > 0:
                        # interleave O-projection of chunk c-1 into this chunk
                        npts = min(4, njt - 1)
                        for gi in range(8):
                            key = 1 + (gi % npts) * (njt - 1) // npts
                            hooks.setdefault(key, []).append(
                                lambda o=gi, cc=c - 1: p4_group(o, cc))
                    p3_chunk(pair, st_cur, c, hooks)
                st_cur = st_next
            for ot in range(D // P):
                p4_group(ot, NCH - 1)

            for c_ in reversed(pair_ctx):
                c_.__exit__(None, None, None)

    nc.compile()
    return nc


def _get_nc():
    global _CACHED_NC
    if _CACHED_NC is None:
        _CACHED_NC = build_nc()
    return _CACHED_NC


def make_in_maps(x, token_positions, Wq, Wk, Wv, Wo):
    x = np.asarray(x, dtype=np.float32)
    Wq = np.asarray(Wq, dtype=np.float32)
    Wk = np.asarray(Wk, dtype=np.float32)
    Wv = np.asarray(Wv, dtype=np.float32)
    Wo = np.asarray(Wo, dtype=np.float32)
    pos = np.asarray(token_positions).astype(np.float64)

    freq_idx = np.arange(0, DK, 2, dtype=np.float64)
    inv_freq = 1.0 / (10000.0 ** (freq_idx / DK))
    ang = pos[:, None] * inv_freq[None, :]          # [S, DK/2]
    cos_t = np.cos(ang).astype(np.float32).T        # [DK/2, S]
    sin_t = np.sin(ang).astype(np.float32).T

    pidx = (np.arange(P) % DK) // 2
    cosn = np.ascontiguousarray(cos_t[pidx, :])     # [128, S]
    sgn = np.where(np.arange(P) % 2 == 0, -1.0, 1.0).astype(np.float32)
    sins = np.ascontiguousarray(sin_t[pidx, :] * sgn[:, None])

    psw = np.zeros((P, P), dtype=np.float32)
    psw[np.arange(P), np.arange(P) ^ 1] = 1.0

    in_maps = []
    for core in range(8):
        b, g = core // 2, core % 2
        sl = slice(512 * g, 512 * g + 512)
        in_maps.append({
            "xT": np.ascontiguousarray(x[b].T),
            "wq": np.ascontiguousarray(Wq[sl, :].T),
            "wk": np.ascontiguousarray(Wk[sl, :].T),
            "wv": np.ascontiguousarray(Wv[sl, :].T),
            "wo": np.ascontiguousarray(Wo[:, sl].T),
            "cosn": cosn,
            "sins": sins,
            "psw": psw,
        })
    return in_maps


def kernel(x, token_positions, Wq, Wk, Wv, Wo):
    global LAST_RESULTS
    nc = _get_nc()
    in_maps = make_in_maps(x, token_positions, Wq, Wk, Wv, Wo)
    res = run_bass_kernel_spmd(nc, in_maps, list(range(8)))
    LAST_RESULTS = res
    B = x.shape[0]
    outp = np.empty((B, S, D), dtype=np.float32)
    for b in range(B):
        outp[b] = (res.results[2 * b]["out"] + res.results[2 * b + 1]["out"]).T
    return outp


# revision 23
# speedup vs baseline: 1.1965x; 1.0256x over previous
"""Causal multi-head attention with RoPE on 8 Trainium2 NeuronCores.

Sharding: core c -> batch b = c // 2, head-group g = c % 2 (8 heads each).
Each core computes q/k/v projections for its 512 output dims, RoPE, causal
attention for its 8 heads, and a partial O-projection. Host sums the two
partial outputs per batch and transposes back.

Device layout notes:
  - All matmul operands are bitcast to float32r (full PE rate at N>=256,
    fp32 storage).
  - q/k are kept transposed [d, s] per head-pair tile [128, 2048]
    (head 2p on partitions 0..63, head 2p+1 on 64..127).
  - RoPE: q' = q * cos + swap(q) * sin_signed, where swap is an
    adjacent-partition-pair permutation done with a 128x128 permutation
    matmul; cos/sin tables arrive pre-expanded from the host.
  - Scores are computed transposed (keys on partitions) so softmax
    needs no DVE reductions: exp() goes straight from PSUM through the
    scalar engine, the denominator comes from a ones-column appended to V,
    and causal masking is a post-exp affine_select fill with 0.
  - v is stored naturally [s, d] with per-head interleaved ones columns
    ([128, 8*65] tiles) so PV lhsT slices are contiguous.
"""

import os
import numpy as np

import concourse.bass as bass
import concourse.tile as tile
from concourse import bacc, mybir
from concourse.bass_utils import run_bass_kernel_spmd

F32 = mybir.dt.float32
F32R = mybir.dt.float32r
BF16 = mybir.dt.bfloat16
MULT = mybir.AluOpType.mult
IS_GE = mybir.AluOpType.is_ge
EXP = mybir.ActivationFunctionType.Exp

P = 128          # partitions
S = 2048         # sequence length
D = 1024         # model dim
DK = 64          # head dim
HPC = 8          # heads per core
NPAIR = 4        # head pairs per core
KT = 8           # 128-row k-tiles of the contraction dim (D)
CH = 512         # i-chunk width (f32r wants moving dim >= 256)
NCH = S // CH    # 4 i-chunks
NJT = S // P     # 16 j-tiles

_CACHED_NC = None
LAST_RESULTS = None


def _r32(ap):
    return ap.bitcast(F32R)


def build_nc():
    nc = bacc.Bacc("TRN2", target_bir_lowering=False, debug=False)

    xT = nc.dram_tensor("xT", [D, S], F32, kind="ExternalInput").ap()
    wq = nc.dram_tensor("wq", [D, 512], F32, kind="ExternalInput").ap()
    wk = nc.dram_tensor("wk", [D, 512], F32, kind="ExternalInput").ap()
    wv = nc.dram_tensor("wv", [D, 512], F32, kind="ExternalInput").ap()
    wo = nc.dram_tensor("wo", [512, D], F32, kind="ExternalInput").ap()
    cosn = nc.dram_tensor("cosn", [P, S], F32, kind="ExternalInput").ap()
    sins = nc.dram_tensor("sins", [P, S], F32, kind="ExternalInput").ap()
    psw = nc.dram_tensor("psw", [P, P], F32, kind="ExternalInput").ap()
    out = nc.dram_tensor("out", [D, S], F32, kind="ExternalOutput").ap()

    xT3 = xT.rearrange("(kt p) s -> p kt s", p=P)
    wq3 = wq.rearrange("(kt p) o -> p kt o", p=P)
    wk3 = wk.rearrange("(kt p) o -> p kt o", p=P)
    wv3 = wv.rearrange("(kt p) o -> p kt o", p=P)
    wo3 = wo.rearrange("(pt p) o -> p pt o", p=P)

    with tile.TileContext(nc) as tc:
        with tc.tile_pool(name="persist", bufs=1) as persist:
            cos_sb = persist.tile([P, S], F32, tag="cos")
            sin_sb = persist.tile([P, S], F32, tag="sin")
            psw_sb = persist.tile([P, P], F32R, tag="psw")

            v_sb = [persist.tile([P, HPC * 65], F32R, name=f"v{jt}", tag=f"v{jt}") for jt in range(NJT)]
            ones8 = persist.tile([P, HPC], F32, tag="ones8")
            nc.vector.memset(ones8[:], 1.0)
            # touch Exp early so the ~2.7us ACT table load overlaps DMAs
            nc.scalar.activation(ones8[0:1, :], ones8[0:1, :], EXP, scale=0.0)
            att_sb = [persist.tile([P, S], F32R, name=f"att{p}", tag=f"att{p}") for p in range(NPAIR)]

            # ---- Phase 1: V projection, all heads at once (N=512) ----
            with (
                tc.tile_pool(name="p1w", bufs=1) as p1w,
                tc.tile_pool(name="pp1", bufs=1, space="PSUM") as pp1,
            ):
                wv_sb = p1w.tile([P, KT, 512], F32R, tag="wv")
                nc.sync.dma_start(wv_sb[:, 0:2, :], wv3[:, 0:2, :].bitcast(F32R))
                nc.sync.dma_start(wv_sb[:, 2:KT, :], wv3[:, 2:KT, :].bitcast(F32R))
                xkt = [p1w.tile([P, S], F32R, name=f"xkt{kt}", tag=f"xkt{kt}") for kt in range(KT)]
                for kt in range(KT):
                    nc.sync.dma_start(
                        xkt[kt][:, 0:256], xT3[:, kt, 0:256].bitcast(F32R))
                    nc.sync.dma_start(
                        xkt[kt][:, 256:S], xT3[:, kt, 256:S].bitcast(F32R))
                for st in range(NJT):
                    ps = pp1.tile([P, 512], F32, tag="p1ps", bufs=2)
                    for kt in range(KT):
                        nc.tensor.matmul(
                            ps[:],
                            _r32(xkt[kt][:, st * P:(st + 1) * P]),
                            _r32(wv_sb[:, kt, :]),
                            start=(kt == 0),
                            stop=(kt == KT - 1),
                        )
                    v3 = v_sb[st][:].rearrange("p (h e) -> p h e", e=65)
                    nc.vector.tensor_copy(
                        out=v3[:, :, 0:DK],
                        in_=ps[:].rearrange("p (h d) -> p h d", d=DK),
                    )
                    nc.vector.tensor_copy(
                        out=v3[:, :, DK:65], in_=ones8[:, :, None])

            # ---- Phases 2+3 per head pair ----
            pair_ctx = [
                tc.tile_pool(name="work", bufs=2),
                tc.tile_pool(name="tmp", bufs=2),
                tc.tile_pool(name="expp", bufs=3),
                tc.tile_pool(name="pp23", bufs=1, space="PSUM"),
            ]
            work, tmp, expp, pp = [c.__enter__() for c in pair_ctx]

            def p2_prefetch(pair):
                st = {}
                st["q"] = work.tile([P, S], BF16, tag="qpair", name=f"q{pair}")
                st["k"] = work.tile([P, S], BF16, tag="kpair", name=f"k{pair}")
                st["wq"] = work.tile([P, KT, P], F32R, tag="wqp", name=f"wq{pair}", bufs=1)
                st["wk"] = work.tile([P, KT, P], F32R, tag="wkp", name=f"wk{pair}", bufs=1)
                osl = slice(pair * P, (pair + 1) * P)
                nc.sync.dma_start(st["wq"][:], wq3[:, :, osl].bitcast(F32R))
                nc.sync.dma_start(st["wk"][:], wk3[:, :, osl].bitcast(F32R))
                return st

            def p2_load_x(st, c):
                ssl = slice(c * CH, (c + 1) * CH)
                xsl = work.tile([P, KT, CH], F32R, tag="xsl", bufs=2)
                for kq in range(4):
                    nc.sync.dma_start(
                        xsl[:, 2 * kq:2 * kq + 2, :],
                        xT3[:, 2 * kq:2 * kq + 2, ssl].bitcast(F32R))
                st["xsl"] = xsl

            def p2_proj(st, c, which):
                # one tensor (q or k): 8-matmul projection burst + RoPE
                ssl = slice(c * CH, (c + 1) * CH)
                w_t = st["wq"] if which == "q" else st["wk"]
                dst = st["q"] if which == "q" else st["k"]
                xsl = st["xsl"]
                ps2 = pp.tile([P, 2, CH], F32, tag="ps2", bufs=1)
                for kt in range(KT):
                    nc.tensor.matmul(
                        ps2[:, 0, :], _r32(w_t[:, kt, :]), _r32(xsl[:, kt, :]),
                        start=(kt == 0), stop=(kt == KT - 1))
                raw = tmp.tile([P, CH], F32R, tag="raw")
                nc.scalar.copy(out=raw[:], in_=ps2[:, 0, :])
                nc.tensor.matmul(
                    ps2[:, 1, :], _r32(psw_sb[:]), _r32(raw[:]),
                    start=True, stop=True)
                tsin = tmp.tile([P, CH], F32, tag="tsin")
                nc.vector.tensor_tensor(tsin[:], ps2[:, 1, :], sin_sb[:, ssl], MULT)
                nc.vector.tensor_tensor(dst[:, ssl], raw[:], cos_sb[:, ssl], MULT)
                nc.vector.tensor_add(out=dst[:, ssl], in0=dst[:, ssl], in1=tsin[:])

            def p3_chunk(pair, st, c, hooks):
                # hooks: {jt_index: fn} emitted between jt iterations to
                # interleave next-pair projection bursts into the PE queue
                h0c, h1c = 65 * (2 * pair), 65 * (2 * pair + 1)
                q_sb, k_sb = st["q"], st["k"]
                ssl = slice(c * CH, (c + 1) * CH)
                psA = pp.tile([65, CH], F32, tag="pvA", bufs=1)
                psB = pp.tile([65, CH], F32, tag="pvB", bufs=1)
                njt = 4 * c + 4
                for jt in range(njt):
                    for fn in hooks.get(jt, ()):
                        fn()
                    start = max(0, (jt - 4 * c) * P)
                    w = CH - start
                    jsl = slice(jt * P, (jt + 1) * P)
                    isl = slice(c * CH + start, (c + 1) * CH)
                    sc = pp.tile([P, 2, CH], F32, tag="sc", bufs=2)
                    nc.tensor.matmul(
                        sc[:, 0, start:], k_sb[0:DK, jsl], q_sb[0:DK, isl],
                        start=True, stop=True, tile_position=(0, 0))
                    nc.tensor.matmul(
                        sc[:, 1, start:], k_sb[DK:P, jsl], q_sb[DK:P, isl],
                        start=True, stop=True, tile_position=(DK, 0))
                    ex = expp.tile([P, 2, CH], F32R, tag="exp")
                    nc.scalar.activation(
                        ex[:, :, start:], sc[:, :, start:], EXP, scale=0.125)
                    if jt >= 4 * c:
                        for hq in range(2):
                            nc.gpsimd.affine_select(
                                out=ex[:, hq, start:], in_=ex[:, hq, start:],
                                compare_op=IS_GE, fill=0.0,
                                base=c * CH + start - jt * P,
                                channel_multiplier=-1,
                                pattern=[[1, w]])
                    first, last = (jt == 0), (jt == njt - 1)
                    nc.tensor.matmul(
                        psA[:, start:], v_sb[jt][:, h0c:h0c + 65],
                        ex[:, 0, start:], start=first, stop=last)
                    nc.tensor.matmul(
                        psB[:, start:], v_sb[jt][:, h1c:h1c + 65],
                        ex[:, 1, start:], start=first, stop=last)
                for ps_, hoff in ((psA, 0), (psB, DK)):
                    d0 = tmp.tile([1, CH], F32, tag="d0")
                    nc.vector.tensor_copy(out=d0[:], in_=ps_[DK:DK + 1, :])
                    rcp = tmp.tile([1, CH], F32, tag="rcp")
                    nc.vector.reciprocal_approx_fast(out=rcp[:], in_=d0[:])
                    bc = tmp.tile([DK, CH], F32, tag="bc")
                    nc.gpsimd.partition_broadcast(bc[:], rcp[:], channels=DK)
                    nc.vector.tensor_tensor(
                        att_sb[pair][hoff:hoff + DK, ssl],
                        ps_[0:DK, :], bc[:], MULT)

            wo_box = {}

            def p4_group(ot, c):
                ssl = slice(c * CH, (c + 1) * CH)
                pso = pp.tile([P, CH], F32, tag="ps2", bufs=1)
                for p_ in range(NPAIR):
                    nc.tensor.matmul(
                        pso[:],
                        wo_box["wo"][:, p_, ot * P:(ot + 1) * P],
                        _r32(att_sb[p_][:, ssl]),
                        start=(p_ == 0), stop=(p_ == NPAIR - 1))
                ob = tmp.tile([P, CH], F32, tag="tsin")
                nc.vector.tensor_copy(out=ob[:], in_=pso[:])
                nc.sync.dma_start(out[ot * P:(ot + 1) * P, ssl], ob[:])

            # tables are first needed by the pair-0 RoPE below
            nc.sync.dma_start(psw_sb[:], psw.bitcast(F32R))
            nc.sync.dma_start(cos_sb[:], cosn)
            nc.sync.dma_start(sin_sb[:], sins)

            # prologue: full P2 for pair 0
            st_cur = p2_prefetch(0)
            for c in range(NCH):
                p2_load_x(st_cur, c)
                p2_proj(st_cur, c, "q")
                p2_proj(st_cur, c, "k")
            for pair in range(NPAIR):
                st_next = p2_prefetch(pair + 1) if pair + 1 < NPAIR else None
                if st_next is None:
                    # last pair: O-projection weights reuse the xsl slot
                    wo_box["wo"] = work.tile([P, NPAIR, D], F32R, tag="xsl", name="wo_sb")
                    nc.sync.dma_start(wo_box["wo"][:], wo3.bitcast(F32R))
                for c in range(NCH):
                    hooks = {}
                    njt = 4 * c + 4
                    if st_next is not None:
                        p2_load_x(st_next, c)
                        hooks[njt // 3] = [
                            lambda sn=st_next, cc=c: p2_proj(sn, cc, "q")]
                        hooks[max(njt // 3 + 1, 2 * njt // 3)] = [
                            lambda sn=st_next, cc=c: p2_proj(sn, cc, "k")]
                    elif c > 0:
                        # interleave O-projection of chunk c-1 into this chunk
                        npts = min(4, njt - 1)
                        for gi in range(8):
                            key = 1 + (gi % npts) * (njt - 1) // npts
                            hooks.setdefault(key, []).append(
                                lambda o=gi, cc=c - 1: p4_group(o, cc))
                    p3_chunk(pair, st_cur, c, hooks)
                st_cur = st_next
            for ot in range(D // P):
                p4_group(ot, NCH - 1)

            for c_ in reversed(pair_ctx):
                c_.__exit__(None, None, None)

    nc.compile()
    return nc


def _get_nc():
    global _CACHED_NC
    if _CACHED_NC is None:
        _CACHED_NC = build_nc()
    return _CACHED_NC


def make_in_maps(x, token_positions, Wq, Wk, Wv, Wo):
    x = np.asarray(x, dtype=np.float32)
    Wq = np.asarray(Wq, dtype=np.float32)
    Wk = np.asarray(Wk, dtype=np.float32)
    Wv = np.asarray(Wv, dtype=np.float32)
    Wo = np.asarray(Wo, dtype=np.float32)
    pos = np.asarray(token_positions).astype(np.float64)

    freq_idx = np.arange(0, DK, 2, dtype=np.float64)
    inv_freq = 1.0 / (10000.0 ** (freq_idx / DK))
    ang = pos[:, None] * inv_freq[None, :]          # [S, DK/2]
    cos_t = np.cos(ang).astype(np.float32).T        # [DK/2, S]
    sin_t = np.sin(ang).astype(np.float32).T

    pidx = (np.arange(P) % DK) // 2
    cosn = np.ascontiguousarray(cos_t[pidx, :])     # [128, S]
    sgn = np.where(np.arange(P) % 2 == 0, -1.0, 1.0).astype(np.float32)
    sins = np.ascontiguousarray(sin_t[pidx, :] * sgn[:, None])

    psw = np.zeros((P, P), dtype=np.float32)
    psw[np.arange(P), np.arange(P) ^ 1] = 1.0

    in_maps = []
    for core in range(8):
        b, g = core // 2, core % 2
        sl = slice(512 * g, 512 * g + 512)
        in_maps.append({
            "xT": np.ascontiguousarray(x[b].T),
            "wq": np.ascontiguousarray(Wq[sl, :].T),
            "wk": np.ascontiguousarray(Wk[sl, :].T),
            "wv": np.ascontiguousarray(Wv[sl, :].T),
            "wo": np.ascontiguousarray(Wo[:, sl].T),
            "cosn": cosn,
            "sins": sins,
            "psw": psw,
        })
    return in_maps


def kernel(x, token_positions, Wq, Wk, Wv, Wo):
    global LAST_RESULTS
    nc = _get_nc()
    in_maps = make_in_maps(x, token_positions, Wq, Wk, Wv, Wo)
    res = run_bass_kernel_spmd(nc, in_maps, list(range(8)))
    LAST_RESULTS = res
    B = x.shape[0]
    outp = np.empty((B, S, D), dtype=np.float32)
    for b in range(B):
        outp[b] = (res.results[2 * b]["out"] + res.results[2 * b + 1]["out"]).T
    return outp


# revision 25
# speedup vs baseline: 1.1978x; 1.0011x over previous
"""Causal multi-head attention with RoPE on 8 Trainium2 NeuronCores.

Sharding: core c -> batch b = c // 2, head-group g = c % 2 (8 heads each).
Each core computes q/k/v projections for its 512 output dims, RoPE, causal
attention for its 8 heads, and a partial O-projection. Host sums the two
partial outputs per batch and transposes back.

Device layout notes:
  - All matmul operands are bitcast to float32r (full PE rate at N>=256,
    fp32 storage).
  - q/k are kept transposed [d, s] per head-pair tile [128, 2048]
    (head 2p on partitions 0..63, head 2p+1 on 64..127).
  - RoPE: q' = q * cos + swap(q) * sin_signed, where swap is an
    adjacent-partition-pair permutation done with a 128x128 permutation
    matmul; cos/sin tables arrive pre-expanded from the host.
  - Scores are computed transposed (keys on partitions) so softmax
    needs no DVE reductions: exp() goes straight from PSUM through the
    scalar engine, the denominator comes from a ones-column appended to V,
    and causal masking is a post-exp affine_select fill with 0.
  - v is stored naturally [s, d] with per-head interleaved ones columns
    ([128, 8*65] tiles) so PV lhsT slices are contiguous.
"""

import os
import numpy as np

import concourse.bass as bass
import concourse.tile as tile
from concourse import bacc, mybir
from concourse.bass_utils import run_bass_kernel_spmd

F32 = mybir.dt.float32
F32R = mybir.dt.float32r
BF16 = mybir.dt.bfloat16
MULT = mybir.AluOpType.mult
IS_GE = mybir.AluOpType.is_ge
EXP = mybir.ActivationFunctionType.Exp

P = 128          # partitions
S = 2048         # sequence length
D = 1024         # model dim
DK = 64          # head dim
HPC = 8          # heads per core
NPAIR = 4        # head pairs per core
KT = 8           # 128-row k-tiles of the contraction dim (D)
CH = 512         # i-chunk width (f32r wants moving dim >= 256)
NCH = S // CH    # 4 i-chunks
NJT = S // P     # 16 j-tiles

_CACHED_NC = None
LAST_RESULTS = None


def _r32(ap):
    return ap.bitcast(F32R)


def build_nc():
    nc = bacc.Bacc("TRN2", target_bir_lowering=False, debug=False)

    xT = nc.dram_tensor("xT", [D, S], F32, kind="ExternalInput").ap()
    wq = nc.dram_tensor("wq", [D, 512], F32, kind="ExternalInput").ap()
    wk = nc.dram_tensor("wk", [D, 512], F32, kind="ExternalInput").ap()
    wv = nc.dram_tensor("wv", [D, 512], F32, kind="ExternalInput").ap()
    wo = nc.dram_tensor("wo", [512, D], F32, kind="ExternalInput").ap()
    cosn = nc.dram_tensor("cosn", [P, S], F32, kind="ExternalInput").ap()
    sins = nc.dram_tensor("sins", [P, S], F32, kind="ExternalInput").ap()
    psw = nc.dram_tensor("psw", [P, P], F32, kind="ExternalInput").ap()
    out = nc.dram_tensor("out", [D, S], F32, kind="ExternalOutput").ap()

    xT3 = xT.rearrange("(kt p) s -> p kt s", p=P)
    wq3 = wq.rearrange("(kt p) o -> p kt o", p=P)
    wk3 = wk.rearrange("(kt p) o -> p kt o", p=P)
    wv3 = wv.rearrange("(kt p) o -> p kt o", p=P)
    wo3 = wo.rearrange("(pt p) o -> p pt o", p=P)

    with tile.TileContext(nc) as tc:
        with tc.tile_pool(name="persist", bufs=1) as persist:
            cos_sb = persist.tile([P, S], F32, tag="cos")
            sin_sb = persist.tile([P, S], F32, tag="sin")
            psw_sb = persist.tile([P, P], F32R, tag="psw")

            v_sb = [persist.tile([P, HPC * 65], F32R, name=f"v{jt}", tag=f"v{jt}") for jt in range(NJT)]
            ones8 = persist.tile([P, HPC], F32, tag="ones8")
            nc.vector.memset(ones8[:], 1.0)
            # touch Exp early so the ~2.7us ACT table load overlaps DMAs
            nc.scalar.activation(ones8[0:1, :], ones8[0:1, :], EXP, scale=0.0)
            att_sb = [persist.tile([P, S], F32R, name=f"att{p}", tag=f"att{p}") for p in range(NPAIR)]

            # ---- Phase 1: V projection, all heads at once (N=512) ----
            with (
                tc.tile_pool(name="p1w", bufs=1) as p1w,
                tc.tile_pool(name="pp1", bufs=1, space="PSUM") as pp1,
            ):
                wv_sb = p1w.tile([P, KT, 512], F32R, tag="wv")
                nc.sync.dma_start(wv_sb[:, 0:2, :], wv3[:, 0:2, :].bitcast(F32R))
                nc.sync.dma_start(wv_sb[:, 2:KT, :], wv3[:, 2:KT, :].bitcast(F32R))
                xkt = [p1w.tile([P, S], F32R, name=f"xkt{kt}", tag=f"xkt{kt}") for kt in range(KT)]
                for kt in range(KT):
                    nc.sync.dma_start(
                        xkt[kt][:, 0:256], xT3[:, kt, 0:256].bitcast(F32R))
                    nc.sync.dma_start(
                        xkt[kt][:, 256:S], xT3[:, kt, 256:S].bitcast(F32R))
                for st in range(NJT):
                    ps = pp1.tile([P, 512], F32, tag="p1ps", bufs=2)
                    for kt in range(KT):
                        nc.tensor.matmul(
                            ps[:],
                            _r32(xkt[kt][:, st * P:(st + 1) * P]),
                            _r32(wv_sb[:, kt, :]),
                            start=(kt == 0),
                            stop=(kt == KT - 1),
                        )
                    v3 = v_sb[st][:].rearrange("p (h e) -> p h e", e=65)
                    nc.vector.tensor_copy(
                        out=v3[:, :, 0:DK],
                        in_=ps[:].rearrange("p (h d) -> p h d", d=DK),
                    )
                    nc.vector.tensor_copy(
                        out=v3[:, :, DK:65], in_=ones8[:, :, None])

            # ---- Phases 2+3 per head pair ----
            pair_ctx = [
                tc.tile_pool(name="work", bufs=2),
                tc.tile_pool(name="tmp", bufs=2),
                tc.tile_pool(name="expp", bufs=3),
                tc.tile_pool(name="pp23", bufs=1, space="PSUM"),
            ]
            work, tmp, expp, pp = [c.__enter__() for c in pair_ctx]

            def p2_prefetch(pair):
                st = {}
                st["q"] = work.tile([P, S], BF16, tag="qpair", name=f"q{pair}")
                st["k"] = work.tile([P, S], BF16, tag="kpair", name=f"k{pair}")
                st["wq"] = work.tile([P, KT, P], F32R, tag="wqp", name=f"wq{pair}", bufs=1)
                st["wk"] = work.tile([P, KT, P], F32R, tag="wkp", name=f"wk{pair}", bufs=1)
                osl = slice(pair * P, (pair + 1) * P)
                nc.sync.dma_start(st["wq"][:], wq3[:, :, osl].bitcast(F32R))
                nc.sync.dma_start(st["wk"][:], wk3[:, :, osl].bitcast(F32R))
                return st

            def p2_load_x(st, c):
                ssl = slice(c * CH, (c + 1) * CH)
                xsl = work.tile([P, KT, CH], F32R, tag="xsl", bufs=3)
                for kq in range(4):
                    nc.sync.dma_start(
                        xsl[:, 2 * kq:2 * kq + 2, :],
                        xT3[:, 2 * kq:2 * kq + 2, ssl].bitcast(F32R))
                st["xsl"] = xsl

            def p2_proj(st, c, which):
                # one tensor (q or k): 8-matmul projection burst + RoPE
                ssl = slice(c * CH, (c + 1) * CH)
                w_t = st["wq"] if which == "q" else st["wk"]
                dst = st["q"] if which == "q" else st["k"]
                xsl = st["xsl"]
                ps2 = pp.tile([P, 2, CH], F32, tag="ps2", bufs=1)
                for kt in range(KT):
                    nc.tensor.matmul(
                        ps2[:, 0, :], _r32(w_t[:, kt, :]), _r32(xsl[:, kt, :]),
                        start=(kt == 0), stop=(kt == KT - 1))
                raw = tmp.tile([P, CH], F32R, tag="raw")
                nc.scalar.copy(out=raw[:], in_=ps2[:, 0, :])
                nc.tensor.matmul(
                    ps2[:, 1, :], _r32(psw_sb[:]), _r32(raw[:]),
                    start=True, stop=True)
                tsin = tmp.tile([P, CH], F32, tag="tsin")
                nc.vector.tensor_tensor(tsin[:], ps2[:, 1, :], sin_sb[:, ssl], MULT)
                nc.vector.tensor_tensor(dst[:, ssl], raw[:], cos_sb[:, ssl], MULT)
                nc.vector.tensor_add(out=dst[:, ssl], in0=dst[:, ssl], in1=tsin[:])

            def p3_chunk(pair, st, c, hooks):
                # hooks: {jt_index: fn} emitted between jt iterations to
                # interleave next-pair projection bursts into the PE queue
                h0c, h1c = 65 * (2 * pair), 65 * (2 * pair + 1)
                q_sb, k_sb = st["q"], st["k"]
                ssl = slice(c * CH, (c + 1) * CH)
                psA = pp.tile([65, CH], F32, tag="pvA", bufs=1)
                psB = pp.tile([65, CH], F32, tag="pvB", bufs=1)
                njt = 4 * c + 4
                for jt in range(njt):
                    for fn in hooks.get(jt, ()):
                        fn()
                    start = max(0, (jt - 4 * c) * P)
                    w = CH - start
                    jsl = slice(jt * P, (jt + 1) * P)
                    isl = slice(c * CH + start, (c + 1) * CH)
                    sc = pp.tile([P, 2, CH], F32, tag="sc", bufs=2)
                    nc.tensor.matmul(
                        sc[:, 0, start:], k_sb[0:DK, jsl], q_sb[0:DK, isl],
                        start=True, stop=True, tile_position=(0, 0))
                    nc.tensor.matmul(
                        sc[:, 1, start:], k_sb[DK:P, jsl], q_sb[DK:P, isl],
                        start=True, stop=True, tile_position=(DK, 0))
                    ex = expp.tile([P, 2, CH], F32R, tag="exp")
                    nc.scalar.activation(
                        ex[:, :, start:], sc[:, :, start:], EXP, scale=0.125)
                    if jt >= 4 * c:
                        for hq in range(2):
                            nc.gpsimd.affine_select(
                                out=ex[:, hq, start:], in_=ex[:, hq, start:],
                                compare_op=IS_GE, fill=0.0,
                                base=c * CH + start - jt * P,
                                channel_multiplier=-1,
                                pattern=[[1, w]])
                    first, last = (jt == 0), (jt == njt - 1)
                    nc.tensor.matmul(
                        psA[:, start:], v_sb[jt][:, h0c:h0c + 65],
                        ex[:, 0, start:], start=first, stop=last)
                    nc.tensor.matmul(
                        psB[:, start:], v_sb[jt][:, h1c:h1c + 65],
                        ex[:, 1, start:], start=first, stop=last)
                for ps_, hoff in ((psA, 0), (psB, DK)):
                    d0 = tmp.tile([1, CH], F32, tag="d0")
                    nc.vector.tensor_copy(out=d0[:], in_=ps_[DK:DK + 1, :])
                    rcp = tmp.tile([1, CH], F32, tag="rcp")
                    nc.vector.reciprocal_approx_fast(out=rcp[:], in_=d0[:])
                    bc = tmp.tile([DK, CH], F32, tag="bc")
                    nc.gpsimd.partition_broadcast(bc[:], rcp[:], channels=DK)
                    nc.vector.tensor_tensor(
                        att_sb[pair][hoff:hoff + DK, ssl],
                        ps_[0:DK, :], bc[:], MULT)

            wo_box = {}

            def p4_group(ot, c):
                ssl = slice(c * CH, (c + 1) * CH)
                pso = pp.tile([P, CH], F32, tag="ps2", bufs=1)
                for p_ in range(NPAIR):
                    nc.tensor.matmul(
                        pso[:],
                        wo_box["wo"][:, p_, ot * P:(ot + 1) * P],
                        _r32(att_sb[p_][:, ssl]),
                        start=(p_ == 0), stop=(p_ == NPAIR - 1))
                ob = tmp.tile([P, CH], F32, tag="tsin")
                nc.vector.tensor_copy(out=ob[:], in_=pso[:])
                nc.sync.dma_start(out[ot * P:(ot + 1) * P, ssl], ob[:])

            # tables stream in per chunk, after each chunk's x columns
            nc.sync.dma_start(psw_sb[:], psw.bitcast(F32R))

            # prologue: full P2 for pair 0
            st_cur = p2_prefetch(0)
            for c in range(NCH):
                ssl = slice(c * CH, (c + 1) * CH)
                p2_load_x(st_cur, c)
                nc.sync.dma_start(cos_sb[:, ssl], cosn[:, ssl])
                nc.sync.dma_start(sin_sb[:, ssl], sins[:, ssl])
                p2_proj(st_cur, c, "q")
                p2_proj(st_cur, c, "k")
            for pair in range(NPAIR):
                st_next = p2_prefetch(pair + 1) if pair + 1 < NPAIR else None
                if st_next is None:
                    # last pair: O-projection weights reuse the xsl slot
                    wo_box["wo"] = work.tile([P, NPAIR, D], F32R, tag="xsl", name="wo_sb", bufs=3)
                    nc.sync.dma_start(wo_box["wo"][:], wo3.bitcast(F32R))
                if st_next is not None:
                    p2_load_x(st_next, 0)
                    p2_load_x(st_next, 1)
                for c in range(NCH):
                    hooks = {}
                    njt = 4 * c + 4
                    if st_next is not None:
                        hooks[njt // 3] = [
                            lambda sn=st_next, cc=c: p2_proj(sn, cc, "q")]
                        hooks[max(njt // 3 + 1, 2 * njt // 3)] = [
                            lambda sn=st_next, cc=c: p2_proj(sn, cc, "k")]
                    elif c > 0:
                        # interleave O-projection of chunk c-1 into this chunk
                        npts = min(4, njt - 1)
                        for gi in range(8):
                            key = 1 + (gi % npts) * (njt - 1) // npts
                            hooks.setdefault(key, []).append(
                                lambda o=gi, cc=c - 1: p4_group(o, cc))
                    p3_chunk(pair, st_cur, c, hooks)
                    if st_next is not None and c + 2 < NCH:
                        p2_load_x(st_next, c + 2)
                st_cur = st_next
            for ot in range(D // P):
                p4_group(ot, NCH - 1)

            for c_ in reversed(pair_ctx):
                c_.__exit__(None, None, None)

    nc.compile()
    return nc


def _get_nc():
    global _CACHED_NC
    if _CACHED_NC is None:
        _CACHED_NC = build_nc()
    return _CACHED_NC


def make_in_maps(x, token_positions, Wq, Wk, Wv, Wo):
    x = np.asarray(x, dtype=np.float32)
    Wq = np.asarray(Wq, dtype=np.float32)
    Wk = np.asarray(Wk, dtype=np.float32)
    Wv = np.asarray(Wv, dtype=np.float32)
    Wo = np.asarray(Wo, dtype=np.float32)
    pos = np.asarray(token_positions).astype(np.float64)

    freq_idx = np.arange(0, DK, 2, dtype=np.float64)
    inv_freq = 1.0 / (10000.0 ** (freq_idx / DK))
    ang = pos[:, None] * inv_freq[None, :]          # [S, DK/2]
    cos_t = np.cos(ang).astype(np.float32).T        # [DK/2, S]
    sin_t = np.sin(ang).astype(np.float32).T

    pidx = (np.arange(P) % DK) // 2
    cosn = np.ascontiguousarray(cos_t[pidx, :])     # [128, S]
    sgn = np.where(np.arange(P) % 2 == 0, -1.0, 1.0).astype(np.float32)
    sins = np.ascontiguousarray(sin_t[pidx, :] * sgn[:, None])

    psw = np.zeros((P, P), dtype=np.float32)
    psw[np.arange(P), np.arange(P) ^ 1] = 1.0

    in_maps = []
    for core in range(8):
        b, g = core // 2, core % 2
        sl = slice(512 * g, 512 * g + 512)
        in_maps.append({
            "xT": np.ascontiguousarray(x[b].T),
            "wq": np.ascontiguousarray(Wq[sl, :].T),
            "wk": np.ascontiguousarray(Wk[sl, :].T),
            "wv": np.ascontiguousarray(Wv[sl, :].T),
            "wo": np.ascontiguousarray(Wo[:, sl].T),
            "cosn": cosn,
            "sins": sins,
            "psw": psw,
        })
    return in_maps


def kernel(x, token_positions, Wq, Wk, Wv, Wo):
    global LAST_RESULTS
    nc = _get_nc()
    in_maps = make_in_maps(x, token_positions, Wq, Wk, Wv, Wo)
    res = run_bass_kernel_spmd(nc, in_maps, list(range(8)))
    LAST_RESULTS = res
    B = x.shape[0]
    outp = np.empty((B, S, D), dtype=np.float32)
    for b in range(B):
        outp[b] = (res.results[2 * b]["out"] + res.results[2 * b + 1]["out"]).T
    return outp


# revision 26
# speedup vs baseline: 1.2499x; 1.0436x over previous
"""Causal multi-head attention with RoPE on 8 Trainium2 NeuronCores.

Sharding: core c -> batch b = c // 2, head-group g = c % 2 (8 heads each).
Each core computes q/k/v projections for its 512 output dims, RoPE, causal
attention for its 8 heads, and a partial O-projection. Host sums the two
partial outputs per batch and transposes back.

Device layout notes:
  - All matmul operands are bitcast to float32r (full PE rate at N>=256,
    fp32 storage).
  - q/k are kept transposed [d, s] per head-pair tile [128, 2048]
    (head 2p on partitions 0..63, head 2p+1 on 64..127).
  - RoPE: q' = q * cos + swap(q) * sin_signed, where swap is an
    adjacent-partition-pair permutation done with a 128x128 permutation
    matmul; cos/sin tables arrive pre-expanded from the host.
  - Scores are computed transposed (keys on partitions) so softmax
    needs no DVE reductions: exp() goes straight from PSUM through the
    scalar engine, the denominator comes from a ones-column appended to V,
    and causal masking is a post-exp affine_select fill with 0.
  - v is stored naturally [s, d] with per-head interleaved ones columns
    ([128, 8*65] tiles) so PV lhsT slices are contiguous.
"""

import os
import numpy as np

import concourse.bass as bass
import concourse.tile as tile
from concourse import bacc, mybir
from concourse.bass_utils import run_bass_kernel_spmd

F32 = mybir.dt.float32
F32R = mybir.dt.float32r
BF16 = mybir.dt.bfloat16
MULT = mybir.AluOpType.mult
IS_GE = mybir.AluOpType.is_ge
EXP = mybir.ActivationFunctionType.Exp

P = 128          # partitions
S = 2048         # sequence length
D = 1024         # model dim
DK = 64          # head dim
HPC = 8          # heads per core
NPAIR = 4        # head pairs per core
KT = 8           # 128-row k-tiles of the contraction dim (D)
CH = 512         # i-chunk width (f32r wants moving dim >= 256)
NCH = S // CH    # 4 i-chunks
NJT = S // P     # 16 j-tiles

_CACHED_NC = None
LAST_RESULTS = None


def _r32(ap):
    return ap.bitcast(F32R)


def build_nc():
    nc = bacc.Bacc("TRN2", target_bir_lowering=False, debug=False)

    xT = nc.dram_tensor("xT", [D, S], F32, kind="ExternalInput").ap()
    wq = nc.dram_tensor("wq", [D, 512], F32, kind="ExternalInput").ap()
    wk = nc.dram_tensor("wk", [D, 512], F32, kind="ExternalInput").ap()
    wv = nc.dram_tensor("wv", [D, 512], F32, kind="ExternalInput").ap()
    wo = nc.dram_tensor("wo", [512, D], F32, kind="ExternalInput").ap()
    cosn = nc.dram_tensor("cosn", [P, S], F32, kind="ExternalInput").ap()
    sins = nc.dram_tensor("sins", [P, S], F32, kind="ExternalInput").ap()
    psw = nc.dram_tensor("psw", [P, P], F32, kind="ExternalInput").ap()
    out = nc.dram_tensor("out", [D, S], F32, kind="ExternalOutput").ap()

    xT3 = xT.rearrange("(kt p) s -> p kt s", p=P)
    wq3 = wq.rearrange("(kt p) o -> p kt o", p=P)
    wk3 = wk.rearrange("(kt p) o -> p kt o", p=P)
    wv3 = wv.rearrange("(kt p) o -> p kt o", p=P)
    wo3 = wo.rearrange("(pt p) o -> p pt o", p=P)

    with tile.TileContext(nc) as tc:
        with tc.tile_pool(name="persist", bufs=1) as persist:
            cos_sb = persist.tile([P, S], F32, tag="cos")
            sin_sb = persist.tile([P, S], F32, tag="sin")
            psw_sb = persist.tile([P, P], F32R, tag="psw")

            v_sb = [persist.tile([P, HPC * 65], F32R, name=f"v{jt}", tag=f"v{jt}") for jt in range(NJT)]
            ones8 = persist.tile([P, HPC], F32, tag="ones8")
            nc.vector.memset(ones8[:], 1.0)
            # touch Exp early so the ~2.7us ACT table load overlaps DMAs
            nc.scalar.activation(ones8[0:1, :], ones8[0:1, :], EXP, scale=0.0)
            att_sb = [persist.tile([P, S], F32R, name=f"att{p}", tag=f"att{p}") for p in range(NPAIR)]

            # ---- Phase 1: V projection, all heads at once (N=512) ----
            with (
                tc.tile_pool(name="p1w", bufs=1) as p1w,
                tc.tile_pool(name="pp1", bufs=1, space="PSUM") as pp1,
            ):
                wv_sb = p1w.tile([P, KT, 512], F32R, tag="wv")
                nc.sync.dma_start(wv_sb[:, 0:2, :], wv3[:, 0:2, :].bitcast(F32R))
                nc.sync.dma_start(wv_sb[:, 2:KT, :], wv3[:, 2:KT, :].bitcast(F32R))
                xkt = [p1w.tile([P, S], F32R, name=f"xkt{kt}", tag=f"xkt{kt}") for kt in range(KT)]
                for kt in range(KT):
                    nc.sync.dma_start(
                        xkt[kt][:, 0:256], xT3[:, kt, 0:256].bitcast(F32R))
                for kt in range(KT):
                    nc.sync.dma_start(
                        xkt[kt][:, 256:1024], xT3[:, kt, 256:1024].bitcast(F32R))
                for kt in range(KT):
                    nc.sync.dma_start(
                        xkt[kt][:, 1024:S], xT3[:, kt, 1024:S].bitcast(F32R))
                for st in range(NJT):
                    ps = pp1.tile([P, 512], F32, tag="p1ps", bufs=2)
                    for kt in range(KT):
                        nc.tensor.matmul(
                            ps[:],
                            _r32(xkt[kt][:, st * P:(st + 1) * P]),
                            _r32(wv_sb[:, kt, :]),
                            start=(kt == 0),
                            stop=(kt == KT - 1),
                        )
                    v3 = v_sb[st][:].rearrange("p (h e) -> p h e", e=65)
                    nc.vector.tensor_copy(
                        out=v3[:, :, 0:DK],
                        in_=ps[:].rearrange("p (h d) -> p h d", d=DK),
                    )
                    nc.vector.tensor_copy(
                        out=v3[:, :, DK:65], in_=ones8[:, :, None])

            # ---- Phases 2+3 per head pair ----
            pair_ctx = [
                tc.tile_pool(name="work", bufs=2),
                tc.tile_pool(name="tmp", bufs=2),
                tc.tile_pool(name="expp", bufs=3),
                tc.tile_pool(name="pp23", bufs=1, space="PSUM"),
            ]
            work, tmp, expp, pp = [c.__enter__() for c in pair_ctx]

            def p2_prefetch(pair):
                st = {}
                st["q"] = work.tile([P, S], BF16, tag="qpair", name=f"q{pair}")
                st["k"] = work.tile([P, S], BF16, tag="kpair", name=f"k{pair}")
                st["wq"] = work.tile([P, KT, P], F32R, tag="wqp", name=f"wq{pair}", bufs=1)
                st["wk"] = work.tile([P, KT, P], F32R, tag="wkp", name=f"wk{pair}", bufs=1)
                osl = slice(pair * P, (pair + 1) * P)
                nc.sync.dma_start(st["wq"][:], wq3[:, :, osl].bitcast(F32R))
                nc.sync.dma_start(st["wk"][:], wk3[:, :, osl].bitcast(F32R))
                return st

            def p2_load_x(st, c):
                ssl = slice(c * CH, (c + 1) * CH)
                xsl = work.tile([P, KT, CH], F32R, tag="xsl", bufs=2)
                for kq in range(4):
                    nc.sync.dma_start(
                        xsl[:, 2 * kq:2 * kq + 2, :],
                        xT3[:, 2 * kq:2 * kq + 2, ssl].bitcast(F32R))
                st["xsl"] = xsl

            def p2_proj(st, c, which):
                # one tensor (q or k): 8-matmul projection burst + RoPE
                ssl = slice(c * CH, (c + 1) * CH)
                w_t = st["wq"] if which == "q" else st["wk"]
                dst = st["q"] if which == "q" else st["k"]
                xsl = st["xsl"]
                ps2 = pp.tile([P, 2, CH], F32, tag="ps2", bufs=1)
                for kt in range(KT):
                    nc.tensor.matmul(
                        ps2[:, 0, :], _r32(w_t[:, kt, :]), _r32(xsl[:, kt, :]),
                        start=(kt == 0), stop=(kt == KT - 1))
                raw = tmp.tile([P, CH], F32R, tag="raw")
                nc.scalar.copy(out=raw[:], in_=ps2[:, 0, :])
                nc.tensor.matmul(
                    ps2[:, 1, :], _r32(psw_sb[:]), _r32(raw[:]),
                    start=True, stop=True)
                tsin = tmp.tile([P, CH], F32, tag="tsin")
                nc.vector.tensor_tensor(tsin[:], ps2[:, 1, :], sin_sb[:, ssl], MULT)
                nc.vector.tensor_tensor(dst[:, ssl], raw[:], cos_sb[:, ssl], MULT)
                nc.vector.tensor_add(out=dst[:, ssl], in0=dst[:, ssl], in1=tsin[:])

            def p3_chunk(pair, st, c, hooks):
                # hooks: {jt_index: fn} emitted between jt iterations to
                # interleave next-pair projection bursts into the PE queue
                h0c, h1c = 65 * (2 * pair), 65 * (2 * pair + 1)
                q_sb, k_sb = st["q"], st["k"]
                ssl = slice(c * CH, (c + 1) * CH)
                psA = pp.tile([65, CH], F32, tag="pvA", bufs=1)
                psB = pp.tile([65, CH], F32, tag="pvB", bufs=1)
                njt = 4 * c + 4
                for jt in range(njt):
                    for fn in hooks.get(jt, ()):
                        fn()
                    start = max(0, (jt - 4 * c) * P)
                    w = CH - start
                    jsl = slice(jt * P, (jt + 1) * P)
                    isl = slice(c * CH + start, (c + 1) * CH)
                    sc = pp.tile([P, 2, CH], F32, tag="sc", bufs=2)
                    nc.tensor.matmul(
                        sc[:, 0, start:], k_sb[0:DK, jsl], q_sb[0:DK, isl],
                        start=True, stop=True, tile_position=(0, 0))
                    nc.tensor.matmul(
                        sc[:, 1, start:], k_sb[DK:P, jsl], q_sb[DK:P, isl],
                        start=True, stop=True, tile_position=(DK, 0))
                    ex = expp.tile([P, 2, CH], F32R, tag="exp")
                    nc.scalar.activation(
                        ex[:, :, start:], sc[:, :, start:], EXP, scale=0.125)
                    if jt >= 4 * c:
                        for hq in range(2):
                            nc.gpsimd.affine_select(
                                out=ex[:, hq, start:], in_=ex[:, hq, start:],
                                compare_op=IS_GE, fill=0.0,
                                base=c * CH + start - jt * P,
                                channel_multiplier=-1,
                                pattern=[[1, w]])
                    first, last = (jt == 0), (jt == njt - 1)
                    nc.tensor.matmul(
                        psA[:, start:], v_sb[jt][:, h0c:h0c + 65],
                        ex[:, 0, start:], start=first, stop=last)
                    nc.tensor.matmul(
                        psB[:, start:], v_sb[jt][:, h1c:h1c + 65],
                        ex[:, 1, start:], start=first, stop=last)
                for ps_, hoff in ((psA, 0), (psB, DK)):
                    d0 = tmp.tile([1, CH], F32, tag="d0")
                    nc.vector.tensor_copy(out=d0[:], in_=ps_[DK:DK + 1, :])
                    rcp = tmp.tile([1, CH], F32, tag="rcp")
                    nc.vector.reciprocal_approx_fast(out=rcp[:], in_=d0[:])
                    bc = tmp.tile([DK, CH], F32, tag="bc")
                    nc.gpsimd.partition_broadcast(bc[:], rcp[:], channels=DK)
                    nc.vector.tensor_tensor(
                        att_sb[pair][hoff:hoff + DK, ssl],
                        ps_[0:DK, :], bc[:], MULT)

            wo_box = {}

            def p4_group(ot, c):
                ssl = slice(c * CH, (c + 1) * CH)
                pso = pp.tile([P, CH], F32, tag="ps2", bufs=1)
                for p_ in range(NPAIR):
                    nc.tensor.matmul(
                        pso[:],
                        wo_box["wo"][:, p_, ot * P:(ot + 1) * P],
                        _r32(att_sb[p_][:, ssl]),
                        start=(p_ == 0), stop=(p_ == NPAIR - 1))
                ob = tmp.tile([P, CH], F32, tag="tsin")
                nc.vector.tensor_copy(out=ob[:], in_=pso[:])
                nc.sync.dma_start(out[ot * P:(ot + 1) * P, ssl], ob[:])

            # tables stream in per chunk, after each chunk's x columns
            nc.sync.dma_start(psw_sb[:], psw.bitcast(F32R))

            # prologue: full P2 for pair 0
            st_cur = p2_prefetch(0)
            for c in range(NCH):
                ssl = slice(c * CH, (c + 1) * CH)
                p2_load_x(st_cur, c)
                nc.sync.dma_start(cos_sb[:, ssl], cosn[:, ssl])
                nc.sync.dma_start(sin_sb[:, ssl], sins[:, ssl])
                p2_proj(st_cur, c, "q")
                p2_proj(st_cur, c, "k")
            for pair in range(NPAIR):
                st_next = p2_prefetch(pair + 1) if pair + 1 < NPAIR else None
                if st_next is None:
                    # last pair: O-projection weights reuse the xsl slot
                    wo_box["wo"] = work.tile([P, NPAIR, D], F32R, tag="xsl", name="wo_sb")
                    nc.sync.dma_start(wo_box["wo"][:], wo3.bitcast(F32R))
                for c in range(NCH):
                    hooks = {}
                    njt = 4 * c + 4
                    if st_next is not None:
                        p2_load_x(st_next, c)
                        hooks[njt // 3] = [
                            lambda sn=st_next, cc=c: p2_proj(sn, cc, "q")]
                        hooks[max(njt // 3 + 1, 2 * njt // 3)] = [
                            lambda sn=st_next, cc=c: p2_proj(sn, cc, "k")]
                    elif c > 0:
                        # interleave O-projection of chunk c-1 into this chunk
                        npts = min(4, njt - 1)
                        for gi in range(8):
                            key = 1 + (gi % npts) * (njt - 1) // npts
                            hooks.setdefault(key, []).append(
                                lambda o=gi, cc=c - 1: p4_group(o, cc))
                    p3_chunk(pair, st_cur, c, hooks)
                st_cur = st_next
            for ot in range(D // P):
                p4_group(ot, NCH - 1)

            for c_ in reversed(pair_ctx):
                c_.__exit__(None, None, None)

    nc.compile()
    return nc


def _get_nc():
    global _CACHED_NC
    if _CACHED_NC is None:
        _CACHED_NC = build_nc()
    return _CACHED_NC


def make_in_maps(x, token_positions, Wq, Wk, Wv, Wo):
    x = np.asarray(x, dtype=np.float32)
    Wq = np.asarray(Wq, dtype=np.float32)
    Wk = np.asarray(Wk, dtype=np.float32)
    Wv = np.asarray(Wv, dtype=np.float32)
    Wo = np.asarray(Wo, dtype=np.float32)
    pos = np.asarray(token_positions).astype(np.float64)

    freq_idx = np.arange(0, DK, 2, dtype=np.float64)
    inv_freq = 1.0 / (10000.0 ** (freq_idx / DK))
    ang = pos[:, None] * inv_freq[None, :]          # [S, DK/2]
    cos_t = np.cos(ang).astype(np.float32).T        # [DK/2, S]
    sin_t = np.sin(ang).astype(np.float32).T

    pidx = (np.arange(P) % DK) // 2
    cosn = np.ascontiguousarray(cos_t[pidx, :])     # [128, S]
    sgn = np.where(np.arange(P) % 2 == 0, -1.0, 1.0).astype(np.float32)
    sins = np.ascontiguousarray(sin_t[pidx, :] * sgn[:, None])

    psw = np.zeros((P, P), dtype=np.float32)
    psw[np.arange(P), np.arange(P) ^ 1] = 1.0

    in_maps = []
    for core in range(8):
        b, g = core // 2, core % 2
        sl = slice(512 * g, 512 * g + 512)
        in_maps.append({
            "xT": np.ascontiguousarray(x[b].T),
            "wq": np.ascontiguousarray(Wq[sl, :].T),
            "wk": np.ascontiguousarray(Wk[sl, :].T),
            "wv": np.ascontiguousarray(Wv[sl, :].T),
            "wo": np.ascontiguousarray(Wo[:, sl].T),
            "cosn": cosn,
            "sins": sins,
            "psw": psw,
        })
    return in_maps


def kernel(x, token_positions, Wq, Wk, Wv, Wo):
    global LAST_RESULTS
    nc = _get_nc()
    in_maps = make_in_maps(x, token_positions, Wq, Wk, Wv, Wo)
    res = run_bass_kernel_spmd(nc, in_maps, list(range(8)))
    LAST_RESULTS = res
    B = x.shape[0]
    outp = np.empty((B, S, D), dtype=np.float32)
    for b in range(B):
        outp[b] = (res.results[2 * b]["out"] + res.results[2 * b + 1]["out"]).T
    return outp
